# revision 16
# baseline (speedup 1.0000x reference)
"""Trainium2 Bass kernel for nn_MessagePassingGNN (B=8192 graphs, N=9 nodes,
16 edges + 9 self-loops per graph, 4 message-passing steps + GRU, decoder).

Strategy (v2, fp8 edge path):
  - Data-parallel over batch: each of 8 cores gets 1024 graphs.
  - Graphs packed into blocks of 14 (126 nodes, 350 edges incl self-loops);
    gather/scatter are matmuls against host-precomputed one-hot incidence
    matrices stored in fp8.
  - Edge-space heavy matmuls run in fp8e4m3 with DoubleRow perf mode
    (2 k-tiles per instruction, 0.5 cyc/row):
      * gather: the dst-P/src-P halves are the two k-tiles -> one matmul
      * m2: K=256 as 2x128 k-tiles
      * w3: K=256 as 2x128 k-tiles
      * scatter: edge chunks (0:128, 128:256) as k-tiles + single for rest
  - P (x@W1), GRU, encoder, decoder stay bf16 for accuracy (fp8 end-to-end
    numpy sim: edge-fp8 rel err ~0.014 vs harness gate 2e-2).
  - PSUM->SBUF fp8 converts (P, m3) run on Vector (GpSimd cannot touch
    PSUM); activations write fp8 directly; the SBUF-only GRU elementwise
    chain runs on GpSimd as plain tensor-tensor ops.
  - GRU gates use true Sigmoid (same activation table set as Tanh, so no
    table reloads); msg_b3 folded into the GRU input bias host-side.
"""

import numpy as np

try:
    import concourse.bass as bass  # noqa: F401
except Exception:  # pragma: no cover
    import sys

    sys.path.insert(0, "/opt/trn_rl_repo")

import ml_dtypes
import concourse.bass as bass
import concourse.bacc as bacc
import concourse.mybir as mybir
from concourse.bass import MemorySpace
from concourse.bass_utils import run_bass_kernel_spmd
from concourse.tile import TileContext


BF16 = mybir.dt.bfloat16
FP8 = mybir.dt.float8e4
F32 = mybir.dt.float32
NPBF16 = ml_dtypes.bfloat16
NPFP8 = ml_dtypes.float8_e4m3
AF = mybir.ActivationFunctionType
ALU = mybir.AluOpType
DR = mybir.MatmulPerfMode.DoubleRow

N, F_IN, H, MH, STEPS = 9, 15, 128, 256, 4
E_PER = 16
EPG = E_PER + N  # 25 edges per graph incl self-loops
NCORES = 8
GPB = 14  # graphs per full block
NN = GPB * N  # 126 nodes per full block
NE = GPB * EPG  # 350 edges per full block
NEP = NE + 2  # fp8 tiles pad the edge dim to a 4-byte-aligned stride

# bias-pack column map
COL_ENC = 0
COL_B1 = lambda s, c: 1 + 2 * s + c
COL_B2 = lambda s, c: 9 + 2 * s + c
COL_BRZ = lambda s, g: 17 + 2 * s + g  # 0.5*(bi'+bh)[g*128:+128]
COL_BHN = lambda s: 25 + s  # bh[256:384]
COL_BIN = lambda s: 29 + s  # bi'[256:384]
COL_DB1 = lambda c: 33 + c
COL_DB2 = lambda c: 35 + c
COL_DB3 = 37
NBIAS = 38


def _derive(bg):
    nblk = bg // GPB
    tailg = bg - nblk * GPB
    totblk = nblk + (1 if tailg else 0)
    nnode = bg * N
    return dict(bg=bg, nblk=nblk, tailg=tailg, totblk=totblk, nnode=nnode)


CFG_FULL = _derive(1024)

_NC_CACHE = {}


WAVE = 6       # pairs interleaved per wave
SD_BUFS = 14
XP_BUFS = 38
ACT_BUFS = 6
PB_BUFS = 2
PB2_BUFS = 3


def build_nc(cfg, repeat=1):
    key = (cfg["bg"], repeat, WAVE, SD_BUFS, XP_BUFS, ACT_BUFS, PB_BUFS, PB2_BUFS)
    if key in _NC_CACHE:
        return _NC_CACHE[key]
    nblk, tailg, totblk, nnode = (
        cfg["nblk"],
        cfg["tailg"],
        cfg["totblk"],
        cfg["nnode"],
    )
    tnn, tne = tailg * N, tailg * EPG

    nc = bacc.Bacc("TRN2", target_bir_lowering=False, debug=False, num_devices=NCORES)

    obsT_d = nc.dram_tensor("obsT", [F_IN, nnode], BF16, kind="ExternalInput")
    sdt_d = nc.dram_tensor("sdt", [totblk, 128, 2, NEP], FP8, kind="ExternalInput")
    dwt_d = nc.dram_tensor("dwt", [totblk, 3, 128, 128], FP8, kind="ExternalInput")
    encw_d = nc.dram_tensor("encw", [F_IN, H], BF16, kind="ExternalInput")
    w1_d = nc.dram_tensor("w1", [STEPS, 2 * H, MH], BF16, kind="ExternalInput")
    w2_d = nc.dram_tensor("w2", [STEPS, MH, MH], FP8, kind="ExternalInput")
    w3_d = nc.dram_tensor("w3", [STEPS, MH, H], FP8, kind="ExternalInput")
    wi_d = nc.dram_tensor("wi", [STEPS, H, 3 * H], BF16, kind="ExternalInput")
    wh_d = nc.dram_tensor("wh", [STEPS, H, 3 * H], BF16, kind="ExternalInput")
    dw1_d = nc.dram_tensor("dw1", [H, MH], BF16, kind="ExternalInput")
    dw2_d = nc.dram_tensor("dw2", [MH, MH], BF16, kind="ExternalInput")
    dw3_d = nc.dram_tensor("dw3", [MH, 1], BF16, kind="ExternalInput")
    bias_d = nc.dram_tensor("biases", [128, NBIAS], F32, kind="ExternalInput")
    out_d = nc.dram_tensor("out", [1, nnode], F32, kind="ExternalOutput")

    NN2 = 2 * NN

    with TileContext(nc) as tc:
        with (
            tc.tile_pool(name="const", bufs=1) as constp,
            tc.tile_pool(name="sd", bufs=SD_BUFS) as sdp,
            tc.tile_pool(name="dw", bufs=SD_BUFS) as dwp,
            tc.tile_pool(name="xp", bufs=XP_BUFS) as xpp,
            tc.tile_pool(name="eact", bufs=ACT_BUFS) as eactp,
            tc.tile_pool(name="gact", bufs=ACT_BUFS) as gactp,
            tc.tile_pool(name="pb", bufs=PB_BUFS, space=MemorySpace.PSUM) as ppb,
            tc.tile_pool(name="pb2", bufs=PB2_BUFS, space=MemorySpace.PSUM) as ppb2,
        ):
            obs_t = constp.tile([F_IN, nnode], BF16, tag="obs")
            nc.sync.dma_start(obs_t[:], obsT_d[:])
            encw_t = constp.tile([F_IN, H], BF16, tag="encw")
            nc.sync.dma_start(encw_t[:], encw_d[:])
            w1_t = constp.tile([128, STEPS, 2, MH], BF16, tag="w1")
            nc.sync.dma_start(
                w1_t[:], w1_d.rearrange("s (kc p) m -> p s kc m", p=128)
            )
            w2_t = constp.tile([128, STEPS, 2, MH], FP8, tag="w2")
            nc.sync.dma_start(
                w2_t[:], w2_d.rearrange("s (kc p) m -> p s kc m", p=128)
            )
            w3_t = constp.tile([128, STEPS, 2, H], FP8, tag="w3")
            nc.sync.dma_start(
                w3_t[:], w3_d.rearrange("s (kc p) m -> p s kc m", p=128)
            )
            wi_t = constp.tile([128, STEPS, 3 * H], BF16, tag="wi")
            nc.sync.dma_start(wi_t[:], wi_d.rearrange("s p m -> p s m"))
            wh_t = constp.tile([128, STEPS, 3 * H], BF16, tag="wh")
            nc.sync.dma_start(wh_t[:], wh_d.rearrange("s p m -> p s m"))
            dw1_t = constp.tile([128, MH], BF16, tag="dw1")
            nc.sync.dma_start(dw1_t[:], dw1_d[:])
            dw2_t = constp.tile([128, 2, MH], BF16, tag="dw2")
            nc.sync.dma_start(dw2_t[:], dw2_d.rearrange("(kc p) m -> p kc m", p=128))
            dw3_t = constp.tile([128, 2, 1], BF16, tag="dw3")
            nc.sync.dma_start(dw3_t[:], dw3_d.rearrange("(kc p) m -> p kc m", p=128))
            bias_t = constp.tile([128, NBIAS], F32, tag="bias")
            nc.sync.dma_start(bias_t[:], bias_d[:])

            def bcol(c):
                return bias_t[:, c : c + 1]

            tot = cfg["totblk"]
            pairs = [tuple(range(kk, min(kk + 2, tot))) for kk in range(0, tot, 2)]

            def geom(k):
                full = k < nblk
                nn = NN if full else tnn
                ne = NE if full else tne
                ecs = [(0, 128), (128, 128), (256, 94)] if full else [(0, tne)]
                return nn, ne, ecs

            class Ctx:
                pass

            def ph_load(cx):
                cx.sds, cx.dws = [], []
                for bi, k in enumerate(cx.pr):
                    nn, ne, ecs = cx.geos[bi]
                    sd = sdp.tile([128, 2, NEP], FP8, tag="sd", name="sd")
                    if k < nblk:
                        nc.sync.dma_start(sd[:, :, :], sdt_d[k])
                    else:
                        nc.sync.dma_start(
                            sd[:nn, :, :ne], sdt_d[k, :nn, :, :ne]
                        )
                    dwti = dwp.tile([128, 3, 128], FP8, tag="dw", name="dw")
                    nch = len(ecs)
                    nc.sync.dma_start(
                        dwti[:, :nch, :nn],
                        dwt_d[k, :nch, :, :nn].rearrange("c p f -> p c f"),
                    )
                    cx.sds.append(sd)
                    cx.dws.append(dwti)

            def ph_enc(cx):
                penc = ppb.tile([128, 512], F32, tag="pb", name="penc")
                nc.tensor.matmul(
                    penc[:, : cx.npair], encw_t[:, :], obs_t[:, cx.pcols],
                    start=True, stop=True,
                )
                cx.xcur = xpp.tile([128, NN2], BF16, tag="xp", name="x0")
                nc.scalar.activation(
                    cx.xcur[:, : cx.npair], penc[:, : cx.npair], AF.Tanh,
                    bias=bcol(COL_ENC),
                )

            def ph_P(cx, s):
                # P = x @ W1-halves in node space (row layout, bf16), then
                # converted fp32->fp8 into SBUF on gpsimd for the gather.
                cx.psb = eactp.tile(
                    [128, 2, 2, 128], FP8, tag="psb", name="psb"
                )  # [part(node), h, mc, feat]
                for bi in range(len(cx.pr)):
                    nn = cx.geos[bi][0]
                    c0 = NN * bi
                    pq = ppb.tile([128, 512], F32, tag="pb", name="pq")
                    nc.tensor.matmul(
                        pq[:nn, :],
                        cx.xcur[:, c0 : c0 + nn],
                        w1_t[:, s, :, :],
                        start=True, stop=True,
                    )
                    if bi == 0:
                        nc.vector.tensor_copy(
                            cx.psb[:126].rearrange("p h m f -> p (h m f)"),
                            pq[:126, :],
                        )
                    else:
                        cx.psb2 = eactp.tile(
                            [128, 2, 2, 128], FP8, tag="psb2", name="psb2"
                        )
                        nn1 = cx.geos[1][0]
                        nc.vector.tensor_copy(
                            cx.psb2[:nn1].rearrange("p h m f -> p (h m f)"),
                            pq[:nn1, :],
                        )

            def ph_m1(cx, s):
                # gather: one DoubleRow matmul per (mc, blk): k-tiles are the
                # dst-P/src-P halves vs the dst/src one-hots.
                cx.m1sb = eactp.tile(
                    [128, 2, 2, NEP], FP8, tag="m1", name="m1sb"
                )  # [part(feat), mc, blk, edge]
                for mc in range(2):
                    pm = ppb2.tile([128, 1024], F32, tag="pb2", name="pm")
                    for bi in range(len(cx.pr)):
                        nn, ne, _ = cx.geos[bi]
                        kk = nn
                        o = 512 * bi
                        psb = cx.psb if bi == 0 else cx.psb2
                        nc.tensor.matmul(
                            pm[:, o : o + ne],
                            psb[:kk, :, mc, :],
                            cx.sds[bi][:kk, :, :ne],
                            start=True, stop=True,
                            perf_mode=DR,
                        )
                    if cx.uni:
                        ne = cx.geos[0][1]
                        nc.scalar.activation(
                            cx.m1sb[:, mc, :, :ne],
                            pm.rearrange("p (b f) -> p b f", b=2)[:, :, :ne],
                            AF.Tanh, bias=bcol(COL_B1(s, mc)),
                        )
                    else:
                        for bi in range(len(cx.pr)):
                            ne = cx.geos[bi][1]
                            nc.scalar.activation(
                                cx.m1sb[:, mc, bi, :ne],
                                pm[:, 512 * bi : 512 * bi + ne],
                                AF.Tanh, bias=bcol(COL_B1(s, mc)),
                            )

            def ph_m2(cx, s):
                cx.m2sb = eactp.tile([128, 2, 2, NEP], FP8, tag="m2", name="m2sb")
                for mc in range(2):
                    pm = ppb2.tile([128, 1024], F32, tag="pb2", name="pm2")
                    for bi in range(len(cx.pr)):
                        nn, ne, _ = cx.geos[bi]
                        o = 512 * bi
                        nc.tensor.matmul(
                            pm[:, o : o + ne],
                            w2_t[:, s, :, mc * 128 : mc * 128 + 128],
                            cx.m1sb[:, :, bi, :ne],
                            start=True, stop=True,
                            perf_mode=DR,
                        )
                    if cx.uni:
                        ne = cx.geos[0][1]
                        nc.scalar.activation(
                            cx.m2sb[:, mc, :, :ne],
                            pm.rearrange("p (b f) -> p b f", b=2)[:, :, :ne],
                            AF.Tanh, bias=bcol(COL_B2(s, mc)),
                        )
                    else:
                        for bi in range(len(cx.pr)):
                            ne = cx.geos[bi][1]
                            nc.scalar.activation(
                                cx.m2sb[:, mc, bi, :ne],
                                pm[:, 512 * bi : 512 * bi + ne],
                                AF.Tanh, bias=bcol(COL_B2(s, mc)),
                            )

            def ph_w3(cx, s):
                # W3 row-form + scatter into one pair tile:
                # per block bi at 512*bi: m3 chunks [0:384], aggr [384:384+nn]
                cx.m3sb = eactp.tile([128, 2, 3, 128], FP8, tag="m3r", name="m3sb")
                cx.aggp = gactp.tile([128, NN2], BF16, tag="aggr", name="aggp")
                aoff = 0
                for bi in range(len(cx.pr)):
                    nn, _, ecs = cx.geos[bi]
                    pg3 = ppb.tile([128, 512], F32, tag="pb", name="pg3")
                    for ci, (e0, el) in enumerate(ecs):
                        nc.tensor.matmul(
                            pg3[:el, 128 * ci : 128 * ci + 128],
                            cx.m2sb[:, :, bi, e0 : e0 + el],
                            w3_t[:, s, :, :],
                            start=True, stop=True,
                            perf_mode=DR,
                        )
                    nch = len(ecs)
                    nc.vector.tensor_copy(
                        cx.m3sb[:, bi, :nch, :].rearrange("p c f -> p (c f)"),
                        pg3[:, : 128 * nch],
                    )
                    if len(ecs) == 3:
                        # edges 0:256 as two DoubleRow k-tiles, 256:350 single
                        nc.tensor.matmul(
                            pg3[:, 384 : 384 + nn],
                            cx.m3sb[:, bi, 0:2, :],
                            cx.dws[bi][:, 0:2, :nn],
                            start=True, stop=False,
                            perf_mode=DR,
                        )
                        nc.tensor.matmul(
                            pg3[:, 384 : 384 + nn],
                            cx.m3sb[:94, bi, 2, :],
                            cx.dws[bi][:94, 2, :nn],
                            start=False, stop=True,
                        )
                    else:
                        (e0, el) = ecs[0]
                        nc.tensor.matmul(
                            pg3[:, 384 : 384 + nn],
                            cx.m3sb[:el, bi, 0, :],
                            cx.dws[bi][:el, 0, :nn],
                            start=True, stop=True,
                        )
                    nc.vector.tensor_copy(
                        cx.aggp[:, aoff : aoff + nn], pg3[:, 384 : 384 + nn]
                    )
                    aoff += nn

            def ph_gru_mm(grp, s):
                # kind-major emission across the group: consecutive matmuls
                # share their stationary operand so walrus ldw-opt drops the
                # redundant LDWEIGHTS.
                for cx in grp:
                    cx.pgr = ppb.tile([128, 512], F32, tag="pb", name="pgr")
                    cx.pgn = ppb.tile([128, 512], F32, tag="pb", name="pgn")
                # pgr: rz0@0, rz1@npair; pgn: gin@0, ghn@npair
                for g, wt in ((0, wi_t), (0, wh_t), (1, wi_t), (1, wh_t)):
                    for cx in grp:
                        mov = cx.aggp if wt is wi_t else cx.xcur
                        nc.tensor.matmul(
                            cx.pgr[:, g * cx.npair : (g + 1) * cx.npair],
                            wt[:, s, g * 128 : g * 128 + 128],
                            mov[:, : cx.npair],
                            start=(wt is wi_t), stop=(wt is wh_t),
                        )
                for cx in grp:
                    nc.tensor.matmul(
                        cx.pgn[:, : cx.npair],
                        wi_t[:, s, 256:384],
                        cx.aggp[:, : cx.npair],
                        start=True, stop=True,
                    )
                for cx in grp:
                    nc.tensor.matmul(
                        cx.pgn[:, cx.npair : 2 * cx.npair],
                        wh_t[:, s, 256:384],
                        cx.xcur[:, : cx.npair],
                        start=True, stop=True,
                    )

            def ph_gru(cx, s):
                npair = cx.npair
                pgr = cx.pgr
                pgn = cx.pgn
                # r/z via true sigmoid: same act table set as tanh
                # (sigmoid_and_others), so no table reload and the gate
                # algebra becomes pure tensor-tensor ops for GpSimd.
                rg = gactp.tile([128, NN2], BF16, tag="thr", name="rg")
                cx.zg = gactp.tile([128, NN2], BF16, tag="thz", name="zg")
                nc.scalar.activation(
                    rg[:, :npair], pgr[:, :npair], AF.Sigmoid,
                    bias=bcol(COL_BRZ(s, 0)),
                )
                nc.scalar.activation(
                    cx.zg[:, :npair], pgr[:, npair : 2 * npair], AF.Sigmoid,
                    bias=bcol(COL_BRZ(s, 1)),
                )
                hnp = gactp.tile([128, NN2], BF16, tag="hnp", name="hnp")
                nc.vector.tensor_scalar_add(
                    hnp[:, :npair],
                    pgn[:, npair : 2 * npair],
                    bcol(COL_BHN(s)),
                )
                rhn = gactp.tile([128, NN2], BF16, tag="rhn", name="rhn")
                nc.gpsimd.tensor_mul(
                    rhn[:, :npair], rg[:, :npair], hnp[:, :npair]
                )
                cx.tn = gactp.tile([128, NN2], BF16, tag="tn", name="tn")
                nc.vector.scalar_tensor_tensor(
                    cx.tn[:, :npair], pgn[:, :npair],
                    bcol(COL_BIN(s)), rhn[:, :npair],
                    op0=ALU.add, op1=ALU.add,
                )

            def ph_xupd(cx, s):
                npair = cx.npair
                ngate = gactp.tile([128, NN2], BF16, tag="ng", name="ng")
                nc.scalar.activation(ngate[:, :npair], cx.tn[:, :npair], AF.Tanh)
                # x' = n + z*(x-n)
                d_ = gactp.tile([128, NN2], BF16, tag="d", name="d_")
                nc.gpsimd.tensor_sub(
                    d_[:, :npair], cx.xcur[:, :npair], ngate[:, :npair]
                )
                w_ = gactp.tile([128, NN2], BF16, tag="w", name="w_")
                nc.gpsimd.tensor_mul(
                    w_[:, :npair], cx.zg[:, :npair], d_[:, :npair]
                )
                xnxt = xpp.tile([128, NN2], BF16, tag="xp", name="xn")
                nc.gpsimd.tensor_add(
                    xnxt[:, :npair], w_[:, :npair], ngate[:, :npair]
                )
                cx.xcur = xnxt

            def ph_dec1(grp):
                for cx in grp:
                    cx.pd1 = ppb.tile([128, 512], F32, tag="pb", name="pd1")
                    cx.d1sb = gactp.tile([128, 2, NN2], BF16, tag="d1", name="d1sb")
                for mc in range(2):
                    for cx in grp:
                        nc.tensor.matmul(
                            cx.pd1[:, 252 * mc : 252 * mc + cx.npair],
                            dw1_t[:, mc * 128 : mc * 128 + 128],
                            cx.xcur[:, : cx.npair],
                            start=True, stop=True,
                        )
                for cx in grp:
                    for mc in range(2):
                        nc.scalar.activation(
                            cx.d1sb[:, mc, : cx.npair],
                            cx.pd1[:, 252 * mc : 252 * mc + cx.npair],
                            AF.Tanh, bias=bcol(COL_DB1(mc)),
                        )

            def ph_dec2(grp):
                for cx in grp:
                    cx.pd2 = ppb.tile([128, 512], F32, tag="pb", name="pd2")
                    cx.d2sb = gactp.tile([128, 2, NN2], BF16, tag="d2", name="d2sb")
                for mc in range(2):
                    for kc in range(2):
                        for cx in grp:
                            nc.tensor.matmul(
                                cx.pd2[:, 252 * mc : 252 * mc + cx.npair],
                                dw2_t[:, kc, mc * 128 : mc * 128 + 128],
                                cx.d1sb[:, kc, : cx.npair],
                                start=(kc == 0), stop=(kc == 1),
                            )
                for cx in grp:
                    for mc in range(2):
                        nc.scalar.activation(
                            cx.d2sb[:, mc, : cx.npair],
                            cx.pd2[:, 252 * mc : 252 * mc + cx.npair],
                            AF.Tanh, bias=bcol(COL_DB2(mc)),
                        )
                for cx in grp:
                    cx.pd3 = ppb.tile([128, 512], F32, tag="pb", name="pd3")
                for kc in range(2):
                    for cx in grp:
                        nc.tensor.matmul(
                            cx.pd3[:1, : cx.npair], dw3_t[:, kc, :],
                            cx.d2sb[:, kc, : cx.npair],
                            start=(kc == 0), stop=(kc == 1),
                        )
                for cx in grp:
                    outp = gactp.tile([1, NN2], F32, tag="outp", name="outp")
                    nc.vector.tensor_scalar_add(
                        outp[:, : cx.npair], cx.pd3[:1, : cx.npair],
                        bias_t[0:1, COL_DB3 : COL_DB3 + 1],
                    )
                    nc.sync.dma_start(out_d[:, cx.pcols], outp[:1, : cx.npair])

            for _rep in range(repeat):
                allpairs = list(pairs)
                waves = [
                    allpairs[i : i + WAVE] for i in range(0, len(allpairs), WAVE)
                ]
                for wv in waves:
                    cxs = []
                    for pr in wv:
                        cx = Ctx()
                        cx.pr = pr
                        cx.geos = [geom(k) for k in pr]
                        cx.uni = len(pr) == 2 and cx.geos[0] == cx.geos[1]
                        cx.col0 = NN * pr[0]
                        cx.npair = sum(g[0] for g in cx.geos)
                        cx.pcols = slice(cx.col0, cx.col0 + cx.npair)
                        cxs.append(cx)
                    for cx in cxs:
                        ph_load(cx)
                    for cx in cxs:
                        ph_enc(cx)
                    for s in range(STEPS):
                        for ph in (ph_P, ph_m1, ph_m2, ph_w3):
                            for cx in cxs:
                                ph(cx, s)
                        for i in range(0, len(cxs), 2):
                            grp = cxs[i : i + 2]
                            ph_gru_mm(grp, s)
                            for cx in grp:
                                ph_gru(cx, s)
                        for cx in cxs:
                            ph_xupd(cx, s)
                    for i in range(0, len(cxs), 2):
                        ph_dec1(cxs[i : i + 2])
                    for i in range(0, len(cxs), 2):
                        ph_dec2(cxs[i : i + 2])

    nc.compile()
    _NC_CACHE[key] = nc
    return nc


def preprocess(inputs, cfg):
    bg, nblk, tailg, totblk, nnode = (
        cfg["bg"], cfg["nblk"], cfg["tailg"], cfg["totblk"], cfg["nnode"],
    )
    b = bg * NCORES
    obs = np.asarray(inputs["obs"], np.float32)
    edges = np.asarray(inputs["edges"], np.int64)

    # one-hot incidence per graph
    src = edges[:, 0, :]
    dst = edges[:, 1, :]
    loops = np.broadcast_to(np.arange(N, dtype=np.int64), (b, N))
    src_all = np.concatenate([src, loops], 1)  # [b, 25]
    dst_all = np.concatenate([dst, loops], 1)
    nod = np.arange(N, dtype=np.int64)
    Sg = (src_all[:, None, :] == nod[None, :, None]).astype(np.float32)  # [b,9,25]
    Dg = (dst_all[:, None, :] == nod[None, :, None]).astype(np.float32)  # [b,9,25]
    deg = Dg.sum(2)  # [b, 9] >= 1
    Dw = Dg.transpose(0, 2, 1) / deg[:, None, :]  # [b, 25, 9]

    SDt = np.zeros((NCORES, totblk, 128, 2, NEP), NPFP8)
    DWf = np.zeros((NCORES, totblk, 384, 128), np.float32)
    Sg_ = Sg.reshape(NCORES, bg, N, EPG)
    Dg_ = Dg.reshape(NCORES, bg, N, EPG)
    Dw_ = Dw.reshape(NCORES, bg, EPG, N)
    nmain = nblk * GPB
    Sm = Sg_[:, :nmain].reshape(NCORES, nblk, GPB, N, EPG)
    Dm = Dg_[:, :nmain].reshape(NCORES, nblk, GPB, N, EPG)
    Wm = Dw_[:, :nmain].reshape(NCORES, nblk, GPB, EPG, N)
    for i in range(GPB):
        r = slice(N * i, N * i + N)
        c = slice(EPG * i, EPG * i + EPG)
        SDt[:, :nblk, r, 0, c] = Dm[:, :, i]  # dst-gather one-hot
        SDt[:, :nblk, r, 1, c] = Sm[:, :, i]  # src-gather one-hot
        DWf[:, :nblk, c, r] = Wm[:, :, i]
    for i in range(tailg):
        g = nmain + i
        r = slice(N * i, N * i + N)
        c = slice(EPG * i, EPG * i + EPG)
        SDt[:, nblk, r, 0, c] = Dg_[:, g]
        SDt[:, nblk, r, 1, c] = Sg_[:, g]
        DWf[:, nblk, c, r] = Dw_[:, g]
    DWt = DWf.reshape(NCORES, totblk, 3, 128, 128).astype(NPFP8)

    obsT = (
        obs.reshape(b, N, F_IN)
        .reshape(NCORES, nnode, F_IN)
        .transpose(0, 2, 1)
        .astype(NPBF16)
    )  # [8, 15, nnode]

    f32 = lambda x: np.asarray(x, np.float32)
    bf = lambda x: np.ascontiguousarray(f32(x)).astype(NPBF16)
    f8 = lambda x: np.ascontiguousarray(f32(x)).astype(NPFP8)

    biases = np.zeros((128, NBIAS), np.float32)
    biases[:, COL_ENC] = f32(inputs["enc_b"])
    gru_bi = f32(inputs["gru_bi"])
    gru_bh = f32(inputs["gru_bh"])
    msg_b3 = f32(inputs["msg_b3"])
    gru_Wi = f32(inputs["gru_Wi"])
    for s in range(STEPS):
        b1 = f32(inputs["msg_b1"][s])
        b2 = f32(inputs["msg_b2"][s])
        for c in range(2):
            biases[:, COL_B1(s, c)] = b1[128 * c : 128 * (c + 1)]
            biases[:, COL_B2(s, c)] = b2[128 * c : 128 * (c + 1)]
        bip = gru_bi[s] + msg_b3[s] @ gru_Wi[s]  # fold msg_b3 into GRU input bias
        for g in range(2):
            biases[:, COL_BRZ(s, g)] = (
                bip[128 * g : 128 * (g + 1)] + gru_bh[s][128 * g : 128 * (g + 1)]
            )
        biases[:, COL_BHN(s)] = gru_bh[s][256:384]
        biases[:, COL_BIN(s)] = bip[256:384]
    db1 = f32(inputs["dec_b1"])
    db2 = f32(inputs["dec_b2"])
    for c in range(2):
        biases[:, COL_DB1(c)] = db1[128 * c : 128 * (c + 1)]
        biases[:, COL_DB2(c)] = db2[128 * c : 128 * (c + 1)]
    biases[0, COL_DB3] = float(f32(inputs["dec_b3"])[0])

    shared = dict(
        encw=bf(inputs["enc_W"]),
        w1=bf(inputs["msg_W1"]),
        w2=f8(inputs["msg_W2"]),
        w3=f8(inputs["msg_W3"]),
        wi=bf(inputs["gru_Wi"]),
        wh=bf(inputs["gru_Wh"]),
        dw1=bf(inputs["dec_W1"]),
        dw2=bf(inputs["dec_W2"]),
        dw3=bf(inputs["dec_W3"]),
        biases=biases,
    )
    in_maps = []
    for c in range(NCORES):
        m = dict(shared)
        m["obsT"] = np.ascontiguousarray(obsT[c])
        m["sdt"] = np.ascontiguousarray(SDt[c])
        m["dwt"] = np.ascontiguousarray(DWt[c])
        in_maps.append(m)
    return in_maps


LAST_EXEC_NS = None
TRACE = False


def _run(inputs, cfg):
    global LAST_EXEC_NS
    nc = build_nc(cfg)
    in_maps = preprocess(inputs, cfg)
    res = run_bass_kernel_spmd(
        nc, in_maps, core_ids=list(range(NCORES)), trace=TRACE
    )
    LAST_EXEC_NS = res.exec_time_ns
    bg = cfg["bg"]
    outs = [np.asarray(res.results[c]["out"], np.float32).reshape(bg, N) for c in range(NCORES)]
    full = np.concatenate(outs, 0)  # [B, 9]
    return np.ascontiguousarray(full[:, :8])


def kernel(**inputs) -> np.ndarray:
    return _run(inputs, CFG_FULL)


# revision 17
# speedup vs baseline: 1.2186x; 1.2186x over previous
"""Trainium2 Bass kernel for nn_MessagePassingGNN (B=8192 graphs, N=9 nodes,
16 edges + 9 self-loops per graph, 4 message-passing steps + GRU, decoder).

Strategy (v2, fp8 edge path):
  - Data-parallel over batch: each of 8 cores gets 1024 graphs.
  - Graphs packed into blocks of 14 (126 nodes, 350 edges incl self-loops);
    gather/scatter are matmuls against host-precomputed one-hot incidence
    matrices stored in fp8.
  - Edge-space heavy matmuls run in fp8e4m3 with DoubleRow perf mode
    (2 k-tiles per instruction, 0.5 cyc/row):
      * gather: the dst-P/src-P halves are the two k-tiles -> one matmul
      * m2: K=256 as 2x128 k-tiles
      * w3: K=256 as 2x128 k-tiles
      * scatter: edge chunks (0:128, 128:256) as k-tiles + single for rest
  - P (x@W1), GRU, encoder, decoder stay bf16 for accuracy (fp8 end-to-end
    numpy sim: edge-fp8 rel err ~0.014 vs harness gate 2e-2).
  - PSUM->SBUF fp8 converts (P, m3) run on Vector (GpSimd cannot touch
    PSUM); activations write fp8 directly; the SBUF-only GRU elementwise
    chain runs on GpSimd as plain tensor-tensor ops.
  - GRU gates use true Sigmoid (same activation table set as Tanh, so no
    table reloads); msg_b3 folded into the GRU input bias host-side.
"""

import numpy as np

try:
    import concourse.bass as bass  # noqa: F401
except Exception:  # pragma: no cover
    import sys

    sys.path.insert(0, "/opt/trn_rl_repo")

import ml_dtypes
import concourse.bass as bass
import concourse.bacc as bacc
import concourse.mybir as mybir
from concourse.bass import MemorySpace
from concourse.bass_utils import run_bass_kernel_spmd
from concourse.tile import TileContext


BF16 = mybir.dt.bfloat16
FP8 = mybir.dt.float8e4
F32 = mybir.dt.float32
NPBF16 = ml_dtypes.bfloat16
NPFP8 = ml_dtypes.float8_e4m3
AF = mybir.ActivationFunctionType
ALU = mybir.AluOpType
DR = mybir.MatmulPerfMode.DoubleRow

N, F_IN, H, MH, STEPS = 9, 15, 128, 256, 4
E_PER = 16
EPG = E_PER + N  # 25 edges per graph incl self-loops
NCORES = 8
GPB = 14  # graphs per full block
NN = GPB * N  # 126 nodes per full block
NE = GPB * EPG  # 350 edges per full block
NEP = NE + 2  # fp8 tiles pad the edge dim to a 4-byte-aligned stride

# bias-pack column map
COL_ENC = 0
COL_B1 = lambda s, c: 1 + 2 * s + c
COL_B2 = lambda s, c: 9 + 2 * s + c
COL_BRZ = lambda s, g: 17 + 2 * s + g  # 0.5*(bi'+bh)[g*128:+128]
COL_BHN = lambda s: 25 + s  # bh[256:384]
COL_BIN = lambda s: 29 + s  # bi'[256:384]
COL_DB1 = lambda c: 33 + c
COL_DB2 = lambda c: 35 + c
COL_DB3 = 37
NBIAS = 38


def _derive(bg):
    nblk = bg // GPB
    tailg = bg - nblk * GPB
    totblk = nblk + (1 if tailg else 0)
    nnode = bg * N
    return dict(bg=bg, nblk=nblk, tailg=tailg, totblk=totblk, nnode=nnode)


CFG_FULL = _derive(1024)

_NC_CACHE = {}


WAVE = 8       # pairs interleaved per wave
SD_BUFS = 18
XP_BUFS = 50
ACT_BUFS = 8
PB_BUFS = 4
PB2_BUFS = 2


def build_nc(cfg, repeat=1):
    key = (cfg["bg"], repeat, WAVE, SD_BUFS, XP_BUFS, ACT_BUFS, PB_BUFS, PB2_BUFS)
    if key in _NC_CACHE:
        return _NC_CACHE[key]
    nblk, tailg, totblk, nnode = (
        cfg["nblk"],
        cfg["tailg"],
        cfg["totblk"],
        cfg["nnode"],
    )
    tnn, tne = tailg * N, tailg * EPG

    nc = bacc.Bacc("TRN2", target_bir_lowering=False, debug=False, num_devices=NCORES)

    obsT_d = nc.dram_tensor("obsT", [F_IN, nnode], BF16, kind="ExternalInput")
    sdt_d = nc.dram_tensor("sdt", [totblk, 128, 2, NEP], FP8, kind="ExternalInput")
    dwt_d = nc.dram_tensor("dwt", [totblk, 3, 128, 128], FP8, kind="ExternalInput")
    encw_d = nc.dram_tensor("encw", [F_IN, H], BF16, kind="ExternalInput")
    w1_d = nc.dram_tensor("w1", [STEPS, 2 * H, MH], BF16, kind="ExternalInput")
    w2_d = nc.dram_tensor("w2", [STEPS, MH, MH], FP8, kind="ExternalInput")
    w3_d = nc.dram_tensor("w3", [STEPS, MH, H], FP8, kind="ExternalInput")
    wi_d = nc.dram_tensor("wi", [STEPS, H, 3 * H], BF16, kind="ExternalInput")
    wh_d = nc.dram_tensor("wh", [STEPS, H, 3 * H], BF16, kind="ExternalInput")
    dw1_d = nc.dram_tensor("dw1", [H, MH], BF16, kind="ExternalInput")
    dw2_d = nc.dram_tensor("dw2", [MH, MH], BF16, kind="ExternalInput")
    dw3_d = nc.dram_tensor("dw3", [MH, 1], BF16, kind="ExternalInput")
    bias_d = nc.dram_tensor("biases", [128, NBIAS], F32, kind="ExternalInput")
    out_d = nc.dram_tensor("out", [1, nnode], F32, kind="ExternalOutput")

    NN2 = 2 * NN

    with TileContext(nc) as tc:
        with (
            tc.tile_pool(name="const", bufs=1) as constp,
            tc.tile_pool(name="sd", bufs=SD_BUFS) as sdp,
            tc.tile_pool(name="dw", bufs=SD_BUFS) as dwp,
            tc.tile_pool(name="xp", bufs=XP_BUFS) as xpp,
            tc.tile_pool(name="eact", bufs=ACT_BUFS) as eactp,
            tc.tile_pool(name="gact", bufs=ACT_BUFS) as gactp,
            tc.tile_pool(name="pb", bufs=PB_BUFS, space=MemorySpace.PSUM) as ppb,
            tc.tile_pool(name="pb2", bufs=PB2_BUFS, space=MemorySpace.PSUM) as ppb2,
        ):
            obs_t = constp.tile([F_IN, nnode], BF16, tag="obs")
            nc.sync.dma_start(obs_t[:], obsT_d[:])
            encw_t = constp.tile([F_IN, H], BF16, tag="encw")
            nc.sync.dma_start(encw_t[:], encw_d[:])
            w1_t = constp.tile([128, STEPS, 2, MH], BF16, tag="w1")
            nc.sync.dma_start(
                w1_t[:], w1_d.rearrange("s (kc p) m -> p s kc m", p=128)
            )
            w2_t = constp.tile([128, STEPS, 2, MH], FP8, tag="w2")
            nc.sync.dma_start(
                w2_t[:], w2_d.rearrange("s (kc p) m -> p s kc m", p=128)
            )
            w3_t = constp.tile([128, STEPS, 2, H], FP8, tag="w3")
            nc.sync.dma_start(
                w3_t[:], w3_d.rearrange("s (kc p) m -> p s kc m", p=128)
            )
            wi_t = constp.tile([128, STEPS, 3 * H], BF16, tag="wi")
            nc.sync.dma_start(wi_t[:], wi_d.rearrange("s p m -> p s m"))
            wh_t = constp.tile([128, STEPS, 3 * H], BF16, tag="wh")
            nc.sync.dma_start(wh_t[:], wh_d.rearrange("s p m -> p s m"))
            dw1_t = constp.tile([128, MH], BF16, tag="dw1")
            nc.sync.dma_start(dw1_t[:], dw1_d[:])
            dw2_t = constp.tile([128, 2, MH], BF16, tag="dw2")
            nc.sync.dma_start(dw2_t[:], dw2_d.rearrange("(kc p) m -> p kc m", p=128))
            dw3_t = constp.tile([128, 2, 1], BF16, tag="dw3")
            nc.sync.dma_start(dw3_t[:], dw3_d.rearrange("(kc p) m -> p kc m", p=128))
            bias_t = constp.tile([128, NBIAS], F32, tag="bias")
            nc.sync.dma_start(bias_t[:], bias_d[:])

            def bcol(c):
                return bias_t[:, c : c + 1]

            tot = cfg["totblk"]
            pairs = [tuple(range(kk, min(kk + 2, tot))) for kk in range(0, tot, 2)]

            def geom(k):
                full = k < nblk
                nn = NN if full else tnn
                ne = NE if full else tne
                ecs = [(0, 128), (128, 128), (256, 94)] if full else [(0, tne)]
                return nn, ne, ecs

            class Ctx:
                pass

            def ph_load(cx):
                cx.sds, cx.dws = [], []
                for bi, k in enumerate(cx.pr):
                    nn, ne, ecs = cx.geos[bi]
                    sd = sdp.tile([128, 2, NEP], FP8, tag="sd", name="sd")
                    if k < nblk:
                        nc.sync.dma_start(sd[:, :, :], sdt_d[k])
                    else:
                        nc.sync.dma_start(
                            sd[:nn, :, :ne], sdt_d[k, :nn, :, :ne]
                        )
                    dwti = dwp.tile([128, 3, 128], FP8, tag="dw", name="dw")
                    nch = len(ecs)
                    nc.sync.dma_start(
                        dwti[:, :nch, :nn],
                        dwt_d[k, :nch, :, :nn].rearrange("c p f -> p c f"),
                    )
                    cx.sds.append(sd)
                    cx.dws.append(dwti)

            def ph_enc(cx):
                penc = ppb.tile([128, 512], F32, tag="pb", name="penc")
                nc.tensor.matmul(
                    penc[:, : cx.npair], encw_t[:, :], obs_t[:, cx.pcols],
                    start=True, stop=True,
                )
                cx.xcur = xpp.tile([128, NN2], BF16, tag="xp", name="x0")
                nc.scalar.activation(
                    cx.xcur[:, : cx.npair], penc[:, : cx.npair], AF.Tanh,
                    bias=bcol(COL_ENC),
                )

            def ph_P(cx, s):
                # P = x @ W1-halves in node space (row layout, bf16), then
                # converted fp32->fp8 into SBUF on gpsimd for the gather.
                cx.psb = eactp.tile(
                    [128, 2, 2, 128], FP8, tag="psb", name="psb"
                )  # [part(node), h, mc, feat]
                for bi in range(len(cx.pr)):
                    nn = cx.geos[bi][0]
                    c0 = NN * bi
                    pq = ppb.tile([128, 512], F32, tag="pb", name="pq")
                    nc.tensor.matmul(
                        pq[:nn, :],
                        cx.xcur[:, c0 : c0 + nn],
                        w1_t[:, s, :, :],
                        start=True, stop=True,
                    )
                    if bi == 0:
                        nc.vector.tensor_copy(
                            cx.psb[:126].rearrange("p h m f -> p (h m f)"),
                            pq[:126, :],
                        )
                    else:
                        cx.psb2 = eactp.tile(
                            [128, 2, 2, 128], FP8, tag="psb2", name="psb2"
                        )
                        nn1 = cx.geos[1][0]
                        nc.vector.tensor_copy(
                            cx.psb2[:nn1].rearrange("p h m f -> p (h m f)"),
                            pq[:nn1, :],
                        )

            def ph_m1(cx, s):
                # gather: one DoubleRow matmul per (mc, blk): k-tiles are the
                # dst-P/src-P halves vs the dst/src one-hots.
                cx.m1sb = eactp.tile(
                    [128, 2, 2, NEP], FP8, tag="m1", name="m1sb"
                )  # [part(feat), mc, blk, edge]
                for mc in range(2):
                    pm = ppb2.tile([128, 1024], F32, tag="pb2", name="pm")
                    for bi in range(len(cx.pr)):
                        nn, ne, _ = cx.geos[bi]
                        kk = nn
                        o = 512 * bi
                        psb = cx.psb if bi == 0 else cx.psb2
                        nc.tensor.matmul(
                            pm[:, o : o + ne],
                            psb[:kk, :, mc, :],
                            cx.sds[bi][:kk, :, :ne],
                            start=True, stop=True,
                            perf_mode=DR,
                        )
                    if cx.uni:
                        ne = cx.geos[0][1]
                        nc.scalar.activation(
                            cx.m1sb[:, mc, :, :ne],
                            pm.rearrange("p (b f) -> p b f", b=2)[:, :, :ne],
                            AF.Tanh, bias=bcol(COL_B1(s, mc)),
                        )
                    else:
                        for bi in range(len(cx.pr)):
                            ne = cx.geos[bi][1]
                            nc.scalar.activation(
                                cx.m1sb[:, mc, bi, :ne],
                                pm[:, 512 * bi : 512 * bi + ne],
                                AF.Tanh, bias=bcol(COL_B1(s, mc)),
                            )

            def ph_m2(cx, s):
                cx.m2sb = eactp.tile([128, 2, 2, NEP], FP8, tag="m2", name="m2sb")
                for mc in range(2):
                    pm = ppb2.tile([128, 1024], F32, tag="pb2", name="pm2")
                    for bi in range(len(cx.pr)):
                        nn, ne, _ = cx.geos[bi]
                        o = 512 * bi
                        nc.tensor.matmul(
                            pm[:, o : o + ne],
                            w2_t[:, s, :, mc * 128 : mc * 128 + 128],
                            cx.m1sb[:, :, bi, :ne],
                            start=True, stop=True,
                            perf_mode=DR,
                        )
                    if cx.uni:
                        ne = cx.geos[0][1]
                        nc.scalar.activation(
                            cx.m2sb[:, mc, :, :ne],
                            pm.rearrange("p (b f) -> p b f", b=2)[:, :, :ne],
                            AF.Tanh, bias=bcol(COL_B2(s, mc)),
                        )
                    else:
                        for bi in range(len(cx.pr)):
                            ne = cx.geos[bi][1]
                            nc.scalar.activation(
                                cx.m2sb[:, mc, bi, :ne],
                                pm[:, 512 * bi : 512 * bi + ne],
                                AF.Tanh, bias=bcol(COL_B2(s, mc)),
                            )

            def ph_w3(cx, s):
                # W3 row-form + scatter into one pair tile:
                # per block bi at 512*bi: m3 chunks [0:384], aggr [384:384+nn]
                cx.m3sb = eactp.tile([128, 2, 3, 128], FP8, tag="m3r", name="m3sb")
                cx.aggp = gactp.tile([128, NN2], BF16, tag="aggr", name="aggp")
                aoff = 0
                for bi in range(len(cx.pr)):
                    nn, _, ecs = cx.geos[bi]
                    pg3 = ppb.tile([128, 512], F32, tag="pb", name="pg3")
                    for ci, (e0, el) in enumerate(ecs):
                        nc.tensor.matmul(
                            pg3[:el, 128 * ci : 128 * ci + 128],
                            cx.m2sb[:, :, bi, e0 : e0 + el],
                            w3_t[:, s, :, :],
                            start=True, stop=True,
                            perf_mode=DR,
                        )
                    nch = len(ecs)
                    nc.vector.tensor_copy(
                        cx.m3sb[:, bi, :nch, :].rearrange("p c f -> p (c f)"),
                        pg3[:, : 128 * nch],
                    )
                    if len(ecs) == 3:
                        # edges 0:256 as two DoubleRow k-tiles, 256:350 single
                        nc.tensor.matmul(
                            pg3[:, 384 : 384 + nn],
                            cx.m3sb[:, bi, 0:2, :],
                            cx.dws[bi][:, 0:2, :nn],
                            start=True, stop=False,
                            perf_mode=DR,
                        )
                        nc.tensor.matmul(
                            pg3[:, 384 : 384 + nn],
                            cx.m3sb[:94, bi, 2, :],
                            cx.dws[bi][:94, 2, :nn],
                            start=False, stop=True,
                        )
                    else:
                        (e0, el) = ecs[0]
                        nc.tensor.matmul(
                            pg3[:, 384 : 384 + nn],
                            cx.m3sb[:el, bi, 0, :],
                            cx.dws[bi][:el, 0, :nn],
                            start=True, stop=True,
                        )
                    nc.vector.tensor_copy(
                        cx.aggp[:, aoff : aoff + nn], pg3[:, 384 : 384 + nn]
                    )
                    aoff += nn

            def ph_gru_mm(grp, s):
                # kind-major emission across the group: consecutive matmuls
                # share their stationary operand so walrus ldw-opt drops the
                # redundant LDWEIGHTS.
                for cx in grp:
                    cx.pgr = ppb.tile([128, 512], F32, tag="pb", name="pgr")
                    cx.pgn = ppb.tile([128, 512], F32, tag="pb", name="pgn")
                # pgr: rz0@0, rz1@npair; pgn: gin@0, ghn@npair
                for g, wt in ((0, wi_t), (0, wh_t), (1, wi_t), (1, wh_t)):
                    for cx in grp:
                        mov = cx.aggp if wt is wi_t else cx.xcur
                        nc.tensor.matmul(
                            cx.pgr[:, g * cx.npair : (g + 1) * cx.npair],
                            wt[:, s, g * 128 : g * 128 + 128],
                            mov[:, : cx.npair],
                            start=(wt is wi_t), stop=(wt is wh_t),
                        )
                for cx in grp:
                    nc.tensor.matmul(
                        cx.pgn[:, : cx.npair],
                        wi_t[:, s, 256:384],
                        cx.aggp[:, : cx.npair],
                        start=True, stop=True,
                    )
                for cx in grp:
                    nc.tensor.matmul(
                        cx.pgn[:, cx.npair : 2 * cx.npair],
                        wh_t[:, s, 256:384],
                        cx.xcur[:, : cx.npair],
                        start=True, stop=True,
                    )

            def ph_gru(cx, s):
                npair = cx.npair
                pgr = cx.pgr
                pgn = cx.pgn
                # r/z via true sigmoid: same act table set as tanh
                # (sigmoid_and_others), so no table reload and the gate
                # algebra becomes pure tensor-tensor ops for GpSimd.
                rg = gactp.tile([128, NN2], BF16, tag="thr", name="rg")
                cx.zg = gactp.tile([128, NN2], BF16, tag="thz", name="zg")
                nc.scalar.activation(
                    rg[:, :npair], pgr[:, :npair], AF.Sigmoid,
                    bias=bcol(COL_BRZ(s, 0)),
                )
                nc.scalar.activation(
                    cx.zg[:, :npair], pgr[:, npair : 2 * npair], AF.Sigmoid,
                    bias=bcol(COL_BRZ(s, 1)),
                )
                hnp = gactp.tile([128, NN2], BF16, tag="hnp", name="hnp")
                nc.vector.tensor_scalar_add(
                    hnp[:, :npair],
                    pgn[:, npair : 2 * npair],
                    bcol(COL_BHN(s)),
                )
                rhn = gactp.tile([128, NN2], BF16, tag="rhn", name="rhn")
                nc.gpsimd.tensor_mul(
                    rhn[:, :npair], rg[:, :npair], hnp[:, :npair]
                )
                cx.tn = gactp.tile([128, NN2], BF16, tag="tn", name="tn")
                nc.vector.scalar_tensor_tensor(
                    cx.tn[:, :npair], pgn[:, :npair],
                    bcol(COL_BIN(s)), rhn[:, :npair],
                    op0=ALU.add, op1=ALU.add,
                )

            def ph_xupd(cx, s):
                npair = cx.npair
                ngate = gactp.tile([128, NN2], BF16, tag="ng", name="ng")
                nc.scalar.activation(ngate[:, :npair], cx.tn[:, :npair], AF.Tanh)
                # x' = n + z*(x-n)
                d_ = gactp.tile([128, NN2], BF16, tag="d", name="d_")
                nc.gpsimd.tensor_sub(
                    d_[:, :npair], cx.xcur[:, :npair], ngate[:, :npair]
                )
                w_ = gactp.tile([128, NN2], BF16, tag="w", name="w_")
                nc.gpsimd.tensor_mul(
                    w_[:, :npair], cx.zg[:, :npair], d_[:, :npair]
                )
                xnxt = xpp.tile([128, NN2], BF16, tag="xp", name="xn")
                nc.gpsimd.tensor_add(
                    xnxt[:, :npair], w_[:, :npair], ngate[:, :npair]
                )
                cx.xcur = xnxt

            def ph_dec1(grp):
                for cx in grp:
                    cx.pd1 = ppb.tile([128, 512], F32, tag="pb", name="pd1")
                    cx.d1sb = gactp.tile([128, 2, NN2], BF16, tag="d1", name="d1sb")
                for mc in range(2):
                    for cx in grp:
                        nc.tensor.matmul(
                            cx.pd1[:, 252 * mc : 252 * mc + cx.npair],
                            dw1_t[:, mc * 128 : mc * 128 + 128],
                            cx.xcur[:, : cx.npair],
                            start=True, stop=True,
                        )
                for cx in grp:
                    for mc in range(2):
                        nc.scalar.activation(
                            cx.d1sb[:, mc, : cx.npair],
                            cx.pd1[:, 252 * mc : 252 * mc + cx.npair],
                            AF.Tanh, bias=bcol(COL_DB1(mc)),
                        )

            def ph_dec2(grp):
                for cx in grp:
                    cx.pd2 = ppb.tile([128, 512], F32, tag="pb", name="pd2")
                    cx.d2sb = gactp.tile([128, 2, NN2], BF16, tag="d2", name="d2sb")
                for mc in range(2):
                    for kc in range(2):
                        for cx in grp:
                            nc.tensor.matmul(
                                cx.pd2[:, 252 * mc : 252 * mc + cx.npair],
                                dw2_t[:, kc, mc * 128 : mc * 128 + 128],
                                cx.d1sb[:, kc, : cx.npair],
                                start=(kc == 0), stop=(kc == 1),
                            )
                for cx in grp:
                    for mc in range(2):
                        nc.scalar.activation(
                            cx.d2sb[:, mc, : cx.npair],
                            cx.pd2[:, 252 * mc : 252 * mc + cx.npair],
                            AF.Tanh, bias=bcol(COL_DB2(mc)),
                        )
                for cx in grp:
                    cx.pd3 = ppb.tile([128, 512], F32, tag="pb", name="pd3")
                for kc in range(2):
                    for cx in grp:
                        nc.tensor.matmul(
                            cx.pd3[:1, : cx.npair], dw3_t[:, kc, :],
                            cx.d2sb[:, kc, : cx.npair],
                            start=(kc == 0), stop=(kc == 1),
                        )
                for cx in grp:
                    outp = gactp.tile([1, NN2], F32, tag="outp", name="outp")
                    nc.vector.tensor_scalar_add(
                        outp[:, : cx.npair], cx.pd3[:1, : cx.npair],
                        bias_t[0:1, COL_DB3 : COL_DB3 + 1],
                    )
                    nc.sync.dma_start(out_d[:, cx.pcols], outp[:1, : cx.npair])

            for _rep in range(repeat):
                allpairs = list(pairs)
                waves = [
                    allpairs[i : i + WAVE] for i in range(0, len(allpairs), WAVE)
                ]
                for wv in waves:
                    cxs = []
                    for pr in wv:
                        cx = Ctx()
                        cx.pr = pr
                        cx.geos = [geom(k) for k in pr]
                        cx.uni = len(pr) == 2 and cx.geos[0] == cx.geos[1]
                        cx.col0 = NN * pr[0]
                        cx.npair = sum(g[0] for g in cx.geos)
                        cx.pcols = slice(cx.col0, cx.col0 + cx.npair)
                        cxs.append(cx)
                    for cx in cxs:
                        ph_load(cx)
                    for cx in cxs:
                        ph_enc(cx)
                    for s in range(STEPS):
                        for ph in (ph_P, ph_m1, ph_m2, ph_w3):
                            for cx in cxs:
                                ph(cx, s)
                        for i in range(0, len(cxs), 2):
                            grp = cxs[i : i + 2]
                            ph_gru_mm(grp, s)
                            for cx in grp:
                                ph_gru(cx, s)
                        for cx in cxs:
                            ph_xupd(cx, s)
                    for i in range(0, len(cxs), 2):
                        ph_dec1(cxs[i : i + 2])
                    for i in range(0, len(cxs), 2):
                        ph_dec2(cxs[i : i + 2])

    nc.compile()
    _NC_CACHE[key] = nc
    return nc


def preprocess(inputs, cfg):
    bg, nblk, tailg, totblk, nnode = (
        cfg["bg"], cfg["nblk"], cfg["tailg"], cfg["totblk"], cfg["nnode"],
    )
    b = bg * NCORES
    obs = np.asarray(inputs["obs"], np.float32)
    edges = np.asarray(inputs["edges"], np.int64)

    # one-hot incidence per graph
    src = edges[:, 0, :]
    dst = edges[:, 1, :]
    loops = np.broadcast_to(np.arange(N, dtype=np.int64), (b, N))
    src_all = np.concatenate([src, loops], 1)  # [b, 25]
    dst_all = np.concatenate([dst, loops], 1)
    nod = np.arange(N, dtype=np.int64)
    Sg = (src_all[:, None, :] == nod[None, :, None]).astype(np.float32)  # [b,9,25]
    Dg = (dst_all[:, None, :] == nod[None, :, None]).astype(np.float32)  # [b,9,25]
    deg = Dg.sum(2)  # [b, 9] >= 1
    Dw = Dg.transpose(0, 2, 1) / deg[:, None, :]  # [b, 25, 9]

    SDt = np.zeros((NCORES, totblk, 128, 2, NEP), NPFP8)
    DWf = np.zeros((NCORES, totblk, 384, 128), np.float32)
    Sg_ = Sg.reshape(NCORES, bg, N, EPG)
    Dg_ = Dg.reshape(NCORES, bg, N, EPG)
    Dw_ = Dw.reshape(NCORES, bg, EPG, N)
    nmain = nblk * GPB
    Sm = Sg_[:, :nmain].reshape(NCORES, nblk, GPB, N, EPG)
    Dm = Dg_[:, :nmain].reshape(NCORES, nblk, GPB, N, EPG)
    Wm = Dw_[:, :nmain].reshape(NCORES, nblk, GPB, EPG, N)
    for i in range(GPB):
        r = slice(N * i, N * i + N)
        c = slice(EPG * i, EPG * i + EPG)
        SDt[:, :nblk, r, 0, c] = Dm[:, :, i]  # dst-gather one-hot
        SDt[:, :nblk, r, 1, c] = Sm[:, :, i]  # src-gather one-hot
        DWf[:, :nblk, c, r] = Wm[:, :, i]
    for i in range(tailg):
        g = nmain + i
        r = slice(N * i, N * i + N)
        c = slice(EPG * i, EPG * i + EPG)
        SDt[:, nblk, r, 0, c] = Dg_[:, g]
        SDt[:, nblk, r, 1, c] = Sg_[:, g]
        DWf[:, nblk, c, r] = Dw_[:, g]
    DWt = DWf.reshape(NCORES, totblk, 3, 128, 128).astype(NPFP8)

    obsT = (
        obs.reshape(b, N, F_IN)
        .reshape(NCORES, nnode, F_IN)
        .transpose(0, 2, 1)
        .astype(NPBF16)
    )  # [8, 15, nnode]

    f32 = lambda x: np.asarray(x, np.float32)
    bf = lambda x: np.ascontiguousarray(f32(x)).astype(NPBF16)
    f8 = lambda x: np.ascontiguousarray(f32(x)).astype(NPFP8)

    biases = np.zeros((128, NBIAS), np.float32)
    biases[:, COL_ENC] = f32(inputs["enc_b"])
    gru_bi = f32(inputs["gru_bi"])
    gru_bh = f32(inputs["gru_bh"])
    msg_b3 = f32(inputs["msg_b3"])
    gru_Wi = f32(inputs["gru_Wi"])
    for s in range(STEPS):
        b1 = f32(inputs["msg_b1"][s])
        b2 = f32(inputs["msg_b2"][s])
        for c in range(2):
            biases[:, COL_B1(s, c)] = b1[128 * c : 128 * (c + 1)]
            biases[:, COL_B2(s, c)] = b2[128 * c : 128 * (c + 1)]
        bip = gru_bi[s] + msg_b3[s] @ gru_Wi[s]  # fold msg_b3 into GRU input bias
        for g in range(2):
            biases[:, COL_BRZ(s, g)] = (
                bip[128 * g : 128 * (g + 1)] + gru_bh[s][128 * g : 128 * (g + 1)]
            )
        biases[:, COL_BHN(s)] = gru_bh[s][256:384]
        biases[:, COL_BIN(s)] = bip[256:384]
    db1 = f32(inputs["dec_b1"])
    db2 = f32(inputs["dec_b2"])
    for c in range(2):
        biases[:, COL_DB1(c)] = db1[128 * c : 128 * (c + 1)]
        biases[:, COL_DB2(c)] = db2[128 * c : 128 * (c + 1)]
    biases[0, COL_DB3] = float(f32(inputs["dec_b3"])[0])

    shared = dict(
        encw=bf(inputs["enc_W"]),
        w1=bf(inputs["msg_W1"]),
        w2=f8(inputs["msg_W2"]),
        w3=f8(inputs["msg_W3"]),
        wi=bf(inputs["gru_Wi"]),
        wh=bf(inputs["gru_Wh"]),
        dw1=bf(inputs["dec_W1"]),
        dw2=bf(inputs["dec_W2"]),
        dw3=bf(inputs["dec_W3"]),
        biases=biases,
    )
    in_maps = []
    for c in range(NCORES):
        m = dict(shared)
        m["obsT"] = np.ascontiguousarray(obsT[c])
        m["sdt"] = np.ascontiguousarray(SDt[c])
        m["dwt"] = np.ascontiguousarray(DWt[c])
        in_maps.append(m)
    return in_maps


LAST_EXEC_NS = None
TRACE = False


def _run(inputs, cfg):
    global LAST_EXEC_NS
    nc = build_nc(cfg)
    in_maps = preprocess(inputs, cfg)
    res = run_bass_kernel_spmd(
        nc, in_maps, core_ids=list(range(NCORES)), trace=TRACE
    )
    LAST_EXEC_NS = res.exec_time_ns
    bg = cfg["bg"]
    outs = [np.asarray(res.results[c]["out"], np.float32).reshape(bg, N) for c in range(NCORES)]
    full = np.concatenate(outs, 0)  # [B, 9]
    return np.ascontiguousarray(full[:, :8])


def kernel(**inputs) -> np.ndarray:
    return _run(inputs, CFG_FULL)


# revision 18
# speedup vs baseline: 1.2610x; 1.0348x over previous
"""Trainium2 Bass kernel for nn_MessagePassingGNN (B=8192 graphs, N=9 nodes,
16 edges + 9 self-loops per graph, 4 message-passing steps + GRU, decoder).

Strategy (v2, fp8 edge path):
  - Data-parallel over batch: each of 8 cores gets 1024 graphs.
  - Graphs packed into blocks of 14 (126 nodes, 350 edges incl self-loops);
    gather/scatter are matmuls against host-precomputed one-hot incidence
    matrices stored in fp8.
  - Edge-space heavy matmuls run in fp8e4m3 with DoubleRow perf mode
    (2 k-tiles per instruction, 0.5 cyc/row):
      * gather: the dst-P/src-P halves are the two k-tiles -> one matmul
      * m2: K=256 as 2x128 k-tiles
      * w3: K=256 as 2x128 k-tiles
      * scatter: edge chunks (0:128, 128:256) as k-tiles + single for rest
  - P (x@W1), GRU, encoder, decoder stay bf16 for accuracy (fp8 end-to-end
    numpy sim: edge-fp8 rel err ~0.014 vs harness gate 2e-2).
  - PSUM->SBUF fp8 converts (P, m3) run on Vector (GpSimd cannot touch
    PSUM); activations write fp8 directly; the SBUF-only GRU elementwise
    chain runs on GpSimd as plain tensor-tensor ops.
  - GRU gates use true Sigmoid (same activation table set as Tanh, so no
    table reloads); msg_b3 folded into the GRU input bias host-side.
"""

import numpy as np

try:
    import concourse.bass as bass  # noqa: F401
except Exception:  # pragma: no cover
    import sys

    sys.path.insert(0, "/opt/trn_rl_repo")

import ml_dtypes
import concourse.bass as bass
import concourse.bacc as bacc
import concourse.mybir as mybir
from concourse.bass import MemorySpace
from concourse.bass_utils import run_bass_kernel_spmd
from concourse.tile import TileContext


BF16 = mybir.dt.bfloat16
FP8 = mybir.dt.float8e4
F32 = mybir.dt.float32
NPBF16 = ml_dtypes.bfloat16
NPFP8 = ml_dtypes.float8_e4m3
AF = mybir.ActivationFunctionType
ALU = mybir.AluOpType
DR = mybir.MatmulPerfMode.DoubleRow

N, F_IN, H, MH, STEPS = 9, 15, 128, 256, 4
E_PER = 16
EPG = E_PER + N  # 25 edges per graph incl self-loops
NCORES = 8
GPB = 14  # graphs per full block
NN = GPB * N  # 126 nodes per full block
NE = GPB * EPG  # 350 edges per full block
NEP = NE + 2  # fp8 tiles pad the edge dim to a 4-byte-aligned stride

# bias-pack column map
COL_ENC = 0
COL_B1 = lambda s, c: 1 + 2 * s + c
COL_B2 = lambda s, c: 9 + 2 * s + c
COL_BRZ = lambda s, g: 17 + 2 * s + g  # 0.5*(bi'+bh)[g*128:+128]
COL_BHN = lambda s: 25 + s  # bh[256:384]
COL_BIN = lambda s: 29 + s  # bi'[256:384]
COL_DB1 = lambda c: 33 + c
COL_DB2 = lambda c: 35 + c
COL_DB3 = 37
NBIAS = 38


def _derive(bg):
    nblk = bg // GPB
    tailg = bg - nblk * GPB
    totblk = nblk + (1 if tailg else 0)
    nnode = bg * N
    return dict(bg=bg, nblk=nblk, tailg=tailg, totblk=totblk, nnode=nnode)


CFG_FULL = _derive(1024)

_NC_CACHE = {}


WAVE = 10      # pairs interleaved per wave
SD_BUFS = 22
XP_BUFS = 60
ACT_BUFS = 10
PB_BUFS = 4
PB2_BUFS = 2


def build_nc(cfg, repeat=1):
    key = (cfg["bg"], repeat, WAVE, SD_BUFS, XP_BUFS, ACT_BUFS, PB_BUFS, PB2_BUFS)
    if key in _NC_CACHE:
        return _NC_CACHE[key]
    nblk, tailg, totblk, nnode = (
        cfg["nblk"],
        cfg["tailg"],
        cfg["totblk"],
        cfg["nnode"],
    )
    tnn, tne = tailg * N, tailg * EPG

    nc = bacc.Bacc("TRN2", target_bir_lowering=False, debug=False, num_devices=NCORES)

    obsT_d = nc.dram_tensor("obsT", [F_IN, nnode], BF16, kind="ExternalInput")
    sdt_d = nc.dram_tensor("sdt", [totblk, 128, 2, NEP], FP8, kind="ExternalInput")
    dwt_d = nc.dram_tensor("dwt", [totblk, 3, 128, 128], FP8, kind="ExternalInput")
    encw_d = nc.dram_tensor("encw", [F_IN, H], BF16, kind="ExternalInput")
    w1_d = nc.dram_tensor("w1", [STEPS, 2 * H, MH], BF16, kind="ExternalInput")
    w2_d = nc.dram_tensor("w2", [STEPS, MH, MH], FP8, kind="ExternalInput")
    w3_d = nc.dram_tensor("w3", [STEPS, MH, H], FP8, kind="ExternalInput")
    wi_d = nc.dram_tensor("wi", [STEPS, H, 3 * H], BF16, kind="ExternalInput")
    wh_d = nc.dram_tensor("wh", [STEPS, H, 3 * H], BF16, kind="ExternalInput")
    dw1_d = nc.dram_tensor("dw1", [H, MH], BF16, kind="ExternalInput")
    dw2_d = nc.dram_tensor("dw2", [MH, MH], BF16, kind="ExternalInput")
    dw3_d = nc.dram_tensor("dw3", [MH, 1], BF16, kind="ExternalInput")
    bias_d = nc.dram_tensor("biases", [128, NBIAS], F32, kind="ExternalInput")
    out_d = nc.dram_tensor("out", [1, nnode], F32, kind="ExternalOutput")

    NN2 = 2 * NN

    with TileContext(nc) as tc:
        with (
            tc.tile_pool(name="const", bufs=1) as constp,
            tc.tile_pool(name="sd", bufs=SD_BUFS) as sdp,
            tc.tile_pool(name="dw", bufs=SD_BUFS) as dwp,
            tc.tile_pool(name="xp", bufs=XP_BUFS) as xpp,
            tc.tile_pool(name="eact", bufs=ACT_BUFS) as eactp,
            tc.tile_pool(name="gact", bufs=ACT_BUFS) as gactp,
            tc.tile_pool(name="pb", bufs=PB_BUFS, space=MemorySpace.PSUM) as ppb,
            tc.tile_pool(name="pb2", bufs=PB2_BUFS, space=MemorySpace.PSUM) as ppb2,
        ):
            obs_t = constp.tile([F_IN, nnode], BF16, tag="obs")
            nc.sync.dma_start(obs_t[:], obsT_d[:])
            encw_t = constp.tile([F_IN, H], BF16, tag="encw")
            nc.sync.dma_start(encw_t[:], encw_d[:])
            w1_t = constp.tile([128, STEPS, 2, MH], BF16, tag="w1")
            nc.sync.dma_start(
                w1_t[:], w1_d.rearrange("s (kc p) m -> p s kc m", p=128)
            )
            w2_t = constp.tile([128, STEPS, 2, MH], FP8, tag="w2")
            nc.sync.dma_start(
                w2_t[:], w2_d.rearrange("s (kc p) m -> p s kc m", p=128)
            )
            w3_t = constp.tile([128, STEPS, 2, H], FP8, tag="w3")
            nc.sync.dma_start(
                w3_t[:], w3_d.rearrange("s (kc p) m -> p s kc m", p=128)
            )
            wi_t = constp.tile([128, STEPS, 3 * H], BF16, tag="wi")
            nc.sync.dma_start(wi_t[:], wi_d.rearrange("s p m -> p s m"))
            wh_t = constp.tile([128, STEPS, 3 * H], BF16, tag="wh")
            nc.sync.dma_start(wh_t[:], wh_d.rearrange("s p m -> p s m"))
            dw1_t = constp.tile([128, MH], BF16, tag="dw1")
            nc.sync.dma_start(dw1_t[:], dw1_d[:])
            dw2_t = constp.tile([128, 2, MH], BF16, tag="dw2")
            nc.sync.dma_start(dw2_t[:], dw2_d.rearrange("(kc p) m -> p kc m", p=128))
            dw3_t = constp.tile([128, 2, 1], BF16, tag="dw3")
            nc.sync.dma_start(dw3_t[:], dw3_d.rearrange("(kc p) m -> p kc m", p=128))
            bias_t = constp.tile([128, NBIAS], F32, tag="bias")
            nc.sync.dma_start(bias_t[:], bias_d[:])

            def bcol(c):
                return bias_t[:, c : c + 1]

            tot = cfg["totblk"]
            pairs = [tuple(range(kk, min(kk + 2, tot))) for kk in range(0, tot, 2)]

            def geom(k):
                full = k < nblk
                nn = NN if full else tnn
                ne = NE if full else tne
                ecs = [(0, 128), (128, 128), (256, 94)] if full else [(0, tne)]
                return nn, ne, ecs

            class Ctx:
                pass

            def ph_load(cx):
                cx.sds, cx.dws = [], []
                for bi, k in enumerate(cx.pr):
                    nn, ne, ecs = cx.geos[bi]
                    sd = sdp.tile([128, 2, NEP], FP8, tag="sd", name="sd")
                    if k < nblk:
                        nc.sync.dma_start(sd[:, :, :], sdt_d[k])
                    else:
                        nc.sync.dma_start(
                            sd[:nn, :, :ne], sdt_d[k, :nn, :, :ne]
                        )
                    dwti = dwp.tile([128, 3, 128], FP8, tag="dw", name="dw")
                    nch = len(ecs)
                    nc.sync.dma_start(
                        dwti[:, :nch, :nn],
                        dwt_d[k, :nch, :, :nn].rearrange("c p f -> p c f"),
                    )
                    cx.sds.append(sd)
                    cx.dws.append(dwti)

            def ph_enc(cx):
                penc = ppb.tile([128, 512], F32, tag="pb", name="penc")
                nc.tensor.matmul(
                    penc[:, : cx.npair], encw_t[:, :], obs_t[:, cx.pcols],
                    start=True, stop=True,
                )
                cx.xcur = xpp.tile([128, NN2], BF16, tag="xp", name="x0")
                nc.scalar.activation(
                    cx.xcur[:, : cx.npair], penc[:, : cx.npair], AF.Tanh,
                    bias=bcol(COL_ENC),
                )

            def ph_P(cx, s):
                # P = x @ W1-halves in node space (row layout, bf16), then
                # converted fp32->fp8 into SBUF on gpsimd for the gather.
                cx.psb = eactp.tile(
                    [128, 2, 2, 128], FP8, tag="psb", name="psb"
                )  # [part(node), h, mc, feat]
                for bi in range(len(cx.pr)):
                    nn = cx.geos[bi][0]
                    c0 = NN * bi
                    pq = ppb.tile([128, 512], F32, tag="pb", name="pq")
                    nc.tensor.matmul(
                        pq[:nn, :],
                        cx.xcur[:, c0 : c0 + nn],
                        w1_t[:, s, :, :],
                        start=True, stop=True,
                    )
                    if bi == 0:
                        nc.vector.tensor_copy(
                            cx.psb[:126].rearrange("p h m f -> p (h m f)"),
                            pq[:126, :],
                        )
                    else:
                        cx.psb2 = eactp.tile(
                            [128, 2, 2, 128], FP8, tag="psb2", name="psb2"
                        )
                        nn1 = cx.geos[1][0]
                        nc.vector.tensor_copy(
                            cx.psb2[:nn1].rearrange("p h m f -> p (h m f)"),
                            pq[:nn1, :],
                        )

            def ph_m1(cx, s):
                # gather: one DoubleRow matmul per (mc, blk): k-tiles are the
                # dst-P/src-P halves vs the dst/src one-hots.
                cx.m1sb = eactp.tile(
                    [128, 2, 2, NEP], FP8, tag="m1", name="m1sb"
                )  # [part(feat), mc, blk, edge]
                for mc in range(2):
                    pm = ppb2.tile([128, 1024], F32, tag="pb2", name="pm")
                    for bi in range(len(cx.pr)):
                        nn, ne, _ = cx.geos[bi]
                        kk = nn
                        o = 512 * bi
                        psb = cx.psb if bi == 0 else cx.psb2
                        nc.tensor.matmul(
                            pm[:, o : o + ne],
                            psb[:kk, :, mc, :],
                            cx.sds[bi][:kk, :, :ne],
                            start=True, stop=True,
                            perf_mode=DR,
                        )
                    if cx.uni:
                        ne = cx.geos[0][1]
                        nc.scalar.activation(
                            cx.m1sb[:, mc, :, :ne],
                            pm.rearrange("p (b f) -> p b f", b=2)[:, :, :ne],
                            AF.Tanh, bias=bcol(COL_B1(s, mc)),
                        )
                    else:
                        for bi in range(len(cx.pr)):
                            ne = cx.geos[bi][1]
                            nc.scalar.activation(
                                cx.m1sb[:, mc, bi, :ne],
                                pm[:, 512 * bi : 512 * bi + ne],
                                AF.Tanh, bias=bcol(COL_B1(s, mc)),
                            )

            def ph_m2(cx, s):
                cx.m2sb = eactp.tile([128, 2, 2, NEP], FP8, tag="m2", name="m2sb")
                for mc in range(2):
                    pm = ppb2.tile([128, 1024], F32, tag="pb2", name="pm2")
                    for bi in range(len(cx.pr)):
                        nn, ne, _ = cx.geos[bi]
                        o = 512 * bi
                        nc.tensor.matmul(
                            pm[:, o : o + ne],
                            w2_t[:, s, :, mc * 128 : mc * 128 + 128],
                            cx.m1sb[:, :, bi, :ne],
                            start=True, stop=True,
                            perf_mode=DR,
                        )
                    if cx.uni:
                        ne = cx.geos[0][1]
                        nc.scalar.activation(
                            cx.m2sb[:, mc, :, :ne],
                            pm.rearrange("p (b f) -> p b f", b=2)[:, :, :ne],
                            AF.Tanh, bias=bcol(COL_B2(s, mc)),
                        )
                    else:
                        for bi in range(len(cx.pr)):
                            ne = cx.geos[bi][1]
                            nc.scalar.activation(
                                cx.m2sb[:, mc, bi, :ne],
                                pm[:, 512 * bi : 512 * bi + ne],
                                AF.Tanh, bias=bcol(COL_B2(s, mc)),
                            )

            def ph_w3(cx, s):
                # W3 row-form + scatter into one pair tile:
                # per block bi at 512*bi: m3 chunks [0:384], aggr [384:384+nn]
                cx.m3sb = eactp.tile([128, 2, 3, 128], FP8, tag="m3r", name="m3sb")
                cx.aggp = gactp.tile([128, NN2], BF16, tag="aggr", name="aggp")
                aoff = 0
                for bi in range(len(cx.pr)):
                    nn, _, ecs = cx.geos[bi]
                    pg3 = ppb.tile([128, 512], F32, tag="pb", name="pg3")
                    for ci, (e0, el) in enumerate(ecs):
                        nc.tensor.matmul(
                            pg3[:el, 128 * ci : 128 * ci + 128],
                            cx.m2sb[:, :, bi, e0 : e0 + el],
                            w3_t[:, s, :, :],
                            start=True, stop=True,
                            perf_mode=DR,
                        )
                    nch = len(ecs)
                    nc.vector.tensor_copy(
                        cx.m3sb[:, bi, :nch, :].rearrange("p c f -> p (c f)"),
                        pg3[:, : 128 * nch],
                    )
                    if len(ecs) == 3:
                        # edges 0:256 as two DoubleRow k-tiles, 256:350 single
                        nc.tensor.matmul(
                            pg3[:, 384 : 384 + nn],
                            cx.m3sb[:, bi, 0:2, :],
                            cx.dws[bi][:, 0:2, :nn],
                            start=True, stop=False,
                            perf_mode=DR,
                        )
                        nc.tensor.matmul(
                            pg3[:, 384 : 384 + nn],
                            cx.m3sb[:94, bi, 2, :],
                            cx.dws[bi][:94, 2, :nn],
                            start=False, stop=True,
                        )
                    else:
                        (e0, el) = ecs[0]
                        nc.tensor.matmul(
                            pg3[:, 384 : 384 + nn],
                            cx.m3sb[:el, bi, 0, :],
                            cx.dws[bi][:el, 0, :nn],
                            start=True, stop=True,
                        )
                    nc.vector.tensor_copy(
                        cx.aggp[:, aoff : aoff + nn], pg3[:, 384 : 384 + nn]
                    )
                    aoff += nn

            def ph_gru_mm(grp, s):
                # kind-major emission across the group: consecutive matmuls
                # share their stationary operand so walrus ldw-opt drops the
                # redundant LDWEIGHTS.
                for cx in grp:
                    cx.pgr = ppb.tile([128, 512], F32, tag="pb", name="pgr")
                    cx.pgn = ppb.tile([128, 512], F32, tag="pb", name="pgn")
                # pgr: rz0@0, rz1@npair; pgn: gin@0, ghn@npair
                for g, wt in ((0, wi_t), (0, wh_t), (1, wi_t), (1, wh_t)):
                    for cx in grp:
                        mov = cx.aggp if wt is wi_t else cx.xcur
                        nc.tensor.matmul(
                            cx.pgr[:, g * cx.npair : (g + 1) * cx.npair],
                            wt[:, s, g * 128 : g * 128 + 128],
                            mov[:, : cx.npair],
                            start=(wt is wi_t), stop=(wt is wh_t),
                        )
                for cx in grp:
                    nc.tensor.matmul(
                        cx.pgn[:, : cx.npair],
                        wi_t[:, s, 256:384],
                        cx.aggp[:, : cx.npair],
                        start=True, stop=True,
                    )
                for cx in grp:
                    nc.tensor.matmul(
                        cx.pgn[:, cx.npair : 2 * cx.npair],
                        wh_t[:, s, 256:384],
                        cx.xcur[:, : cx.npair],
                        start=True, stop=True,
                    )

            def ph_gru(cx, s):
                npair = cx.npair
                pgr = cx.pgr
                pgn = cx.pgn
                # r/z via true sigmoid: same act table set as tanh
                # (sigmoid_and_others), so no table reload and the gate
                # algebra becomes pure tensor-tensor ops for GpSimd.
                rg = gactp.tile([128, NN2], BF16, tag="thr", name="rg")
                cx.zg = gactp.tile([128, NN2], BF16, tag="thz", name="zg")
                nc.scalar.activation(
                    rg[:, :npair], pgr[:, :npair], AF.Sigmoid,
                    bias=bcol(COL_BRZ(s, 0)),
                )
                nc.scalar.activation(
                    cx.zg[:, :npair], pgr[:, npair : 2 * npair], AF.Sigmoid,
                    bias=bcol(COL_BRZ(s, 1)),
                )
                hnp = gactp.tile([128, NN2], BF16, tag="hnp", name="hnp")
                nc.vector.tensor_scalar_add(
                    hnp[:, :npair],
                    pgn[:, npair : 2 * npair],
                    bcol(COL_BHN(s)),
                )
                rhn = gactp.tile([128, NN2], BF16, tag="rhn", name="rhn")
                nc.gpsimd.tensor_mul(
                    rhn[:, :npair], rg[:, :npair], hnp[:, :npair]
                )
                cx.tn = gactp.tile([128, NN2], BF16, tag="tn", name="tn")
                nc.vector.scalar_tensor_tensor(
                    cx.tn[:, :npair], pgn[:, :npair],
                    bcol(COL_BIN(s)), rhn[:, :npair],
                    op0=ALU.add, op1=ALU.add,
                )

            def ph_xupd(cx, s):
                npair = cx.npair
                ngate = gactp.tile([128, NN2], BF16, tag="ng", name="ng")
                nc.scalar.activation(ngate[:, :npair], cx.tn[:, :npair], AF.Tanh)
                # x' = n + z*(x-n)
                d_ = gactp.tile([128, NN2], BF16, tag="d", name="d_")
                nc.gpsimd.tensor_sub(
                    d_[:, :npair], cx.xcur[:, :npair], ngate[:, :npair]
                )
                w_ = gactp.tile([128, NN2], BF16, tag="w", name="w_")
                nc.gpsimd.tensor_mul(
                    w_[:, :npair], cx.zg[:, :npair], d_[:, :npair]
                )
                xnxt = xpp.tile([128, NN2], BF16, tag="xp", name="xn")
                nc.gpsimd.tensor_add(
                    xnxt[:, :npair], w_[:, :npair], ngate[:, :npair]
                )
                cx.xcur = xnxt

            def ph_dec1(grp):
                for cx in grp:
                    cx.pd1 = ppb.tile([128, 512], F32, tag="pb", name="pd1")
                    cx.d1sb = gactp.tile([128, 2, NN2], BF16, tag="d1", name="d1sb")
                for mc in range(2):
                    for cx in grp:
                        nc.tensor.matmul(
                            cx.pd1[:, 252 * mc : 252 * mc + cx.npair],
                            dw1_t[:, mc * 128 : mc * 128 + 128],
                            cx.xcur[:, : cx.npair],
                            start=True, stop=True,
                        )
                for cx in grp:
                    for mc in range(2):
                        nc.scalar.activation(
                            cx.d1sb[:, mc, : cx.npair],
                            cx.pd1[:, 252 * mc : 252 * mc + cx.npair],
                            AF.Tanh, bias=bcol(COL_DB1(mc)),
                        )

            def ph_dec2(grp):
                for cx in grp:
                    cx.pd2 = ppb.tile([128, 512], F32, tag="pb", name="pd2")
                    cx.d2sb = gactp.tile([128, 2, NN2], BF16, tag="d2", name="d2sb")
                for mc in range(2):
                    for kc in range(2):
                        for cx in grp:
                            nc.tensor.matmul(
                                cx.pd2[:, 252 * mc : 252 * mc + cx.npair],
                                dw2_t[:, kc, mc * 128 : mc * 128 + 128],
                                cx.d1sb[:, kc, : cx.npair],
                                start=(kc == 0), stop=(kc == 1),
                            )
                for cx in grp:
                    for mc in range(2):
                        nc.scalar.activation(
                            cx.d2sb[:, mc, : cx.npair],
                            cx.pd2[:, 252 * mc : 252 * mc + cx.npair],
                            AF.Tanh, bias=bcol(COL_DB2(mc)),
                        )
                for cx in grp:
                    cx.pd3 = ppb.tile([128, 512], F32, tag="pb", name="pd3")
                for kc in range(2):
                    for cx in grp:
                        nc.tensor.matmul(
                            cx.pd3[:1, : cx.npair], dw3_t[:, kc, :],
                            cx.d2sb[:, kc, : cx.npair],
                            start=(kc == 0), stop=(kc == 1),
                        )
                for cx in grp:
                    outp = gactp.tile([1, NN2], F32, tag="outp", name="outp")
                    nc.vector.tensor_scalar_add(
                        outp[:, : cx.npair], cx.pd3[:1, : cx.npair],
                        bias_t[0:1, COL_DB3 : COL_DB3 + 1],
                    )
                    nc.sync.dma_start(out_d[:, cx.pcols], outp[:1, : cx.npair])

            for _rep in range(repeat):
                allpairs = list(pairs)
                waves = [
                    allpairs[i : i + WAVE] for i in range(0, len(allpairs), WAVE)
                ]
                for wv in waves:
                    cxs = []
                    for pr in wv:
                        cx = Ctx()
                        cx.pr = pr
                        cx.geos = [geom(k) for k in pr]
                        cx.uni = len(pr) == 2 and cx.geos[0] == cx.geos[1]
                        cx.col0 = NN * pr[0]
                        cx.npair = sum(g[0] for g in cx.geos)
                        cx.pcols = slice(cx.col0, cx.col0 + cx.npair)
                        cxs.append(cx)
                    for cx in cxs:
                        ph_load(cx)
                    for cx in cxs:
                        ph_enc(cx)
                    for s in range(STEPS):
                        for ph in (ph_P, ph_m1, ph_m2, ph_w3):
                            for cx in cxs:
                                ph(cx, s)
                        for i in range(0, len(cxs), 2):
                            grp = cxs[i : i + 2]
                            ph_gru_mm(grp, s)
                            for cx in grp:
                                ph_gru(cx, s)
                        for cx in cxs:
                            ph_xupd(cx, s)
                    for i in range(0, len(cxs), 2):
                        ph_dec1(cxs[i : i + 2])
                    for i in range(0, len(cxs), 2):
                        ph_dec2(cxs[i : i + 2])

    nc.compile()
    _NC_CACHE[key] = nc
    return nc


def preprocess(inputs, cfg):
    bg, nblk, tailg, totblk, nnode = (
        cfg["bg"], cfg["nblk"], cfg["tailg"], cfg["totblk"], cfg["nnode"],
    )
    b = bg * NCORES
    obs = np.asarray(inputs["obs"], np.float32)
    edges = np.asarray(inputs["edges"], np.int64)

    # one-hot incidence per graph
    src = edges[:, 0, :]
    dst = edges[:, 1, :]
    loops = np.broadcast_to(np.arange(N, dtype=np.int64), (b, N))
    src_all = np.concatenate([src, loops], 1)  # [b, 25]
    dst_all = np.concatenate([dst, loops], 1)
    nod = np.arange(N, dtype=np.int64)
    Sg = (src_all[:, None, :] == nod[None, :, None]).astype(np.float32)  # [b,9,25]
    Dg = (dst_all[:, None, :] == nod[None, :, None]).astype(np.float32)  # [b,9,25]
    deg = Dg.sum(2)  # [b, 9] >= 1
    Dw = Dg.transpose(0, 2, 1) / deg[:, None, :]  # [b, 25, 9]

    SDt = np.zeros((NCORES, totblk, 128, 2, NEP), NPFP8)
    DWf = np.zeros((NCORES, totblk, 384, 128), np.float32)
    Sg_ = Sg.reshape(NCORES, bg, N, EPG)
    Dg_ = Dg.reshape(NCORES, bg, N, EPG)
    Dw_ = Dw.reshape(NCORES, bg, EPG, N)
    nmain = nblk * GPB
    Sm = Sg_[:, :nmain].reshape(NCORES, nblk, GPB, N, EPG)
    Dm = Dg_[:, :nmain].reshape(NCORES, nblk, GPB, N, EPG)
    Wm = Dw_[:, :nmain].reshape(NCORES, nblk, GPB, EPG, N)
    for i in range(GPB):
        r = slice(N * i, N * i + N)
        c = slice(EPG * i, EPG * i + EPG)
        SDt[:, :nblk, r, 0, c] = Dm[:, :, i]  # dst-gather one-hot
        SDt[:, :nblk, r, 1, c] = Sm[:, :, i]  # src-gather one-hot
        DWf[:, :nblk, c, r] = Wm[:, :, i]
    for i in range(tailg):
        g = nmain + i
        r = slice(N * i, N * i + N)
        c = slice(EPG * i, EPG * i + EPG)
        SDt[:, nblk, r, 0, c] = Dg_[:, g]
        SDt[:, nblk, r, 1, c] = Sg_[:, g]
        DWf[:, nblk, c, r] = Dw_[:, g]
    DWt = DWf.reshape(NCORES, totblk, 3, 128, 128).astype(NPFP8)

    obsT = (
        obs.reshape(b, N, F_IN)
        .reshape(NCORES, nnode, F_IN)
        .transpose(0, 2, 1)
        .astype(NPBF16)
    )  # [8, 15, nnode]

    f32 = lambda x: np.asarray(x, np.float32)
    bf = lambda x: np.ascontiguousarray(f32(x)).astype(NPBF16)
    f8 = lambda x: np.ascontiguousarray(f32(x)).astype(NPFP8)

    biases = np.zeros((128, NBIAS), np.float32)
    biases[:, COL_ENC] = f32(inputs["enc_b"])
    gru_bi = f32(inputs["gru_bi"])
    gru_bh = f32(inputs["gru_bh"])
    msg_b3 = f32(inputs["msg_b3"])
    gru_Wi = f32(inputs["gru_Wi"])
    for s in range(STEPS):
        b1 = f32(inputs["msg_b1"][s])
        b2 = f32(inputs["msg_b2"][s])
        for c in range(2):
            biases[:, COL_B1(s, c)] = b1[128 * c : 128 * (c + 1)]
            biases[:, COL_B2(s, c)] = b2[128 * c : 128 * (c + 1)]
        bip = gru_bi[s] + msg_b3[s] @ gru_Wi[s]  # fold msg_b3 into GRU input bias
        for g in range(2):
            biases[:, COL_BRZ(s, g)] = (
                bip[128 * g : 128 * (g + 1)] + gru_bh[s][128 * g : 128 * (g + 1)]
            )
        biases[:, COL_BHN(s)] = gru_bh[s][256:384]
        biases[:, COL_BIN(s)] = bip[256:384]
    db1 = f32(inputs["dec_b1"])
    db2 = f32(inputs["dec_b2"])
    for c in range(2):
        biases[:, COL_DB1(c)] = db1[128 * c : 128 * (c + 1)]
        biases[:, COL_DB2(c)] = db2[128 * c : 128 * (c + 1)]
    biases[0, COL_DB3] = float(f32(inputs["dec_b3"])[0])

    shared = dict(
        encw=bf(inputs["enc_W"]),
        w1=bf(inputs["msg_W1"]),
        w2=f8(inputs["msg_W2"]),
        w3=f8(inputs["msg_W3"]),
        wi=bf(inputs["gru_Wi"]),
        wh=bf(inputs["gru_Wh"]),
        dw1=bf(inputs["dec_W1"]),
        dw2=bf(inputs["dec_W2"]),
        dw3=bf(inputs["dec_W3"]),
        biases=biases,
    )
    in_maps = []
    for c in range(NCORES):
        m = dict(shared)
        m["obsT"] = np.ascontiguousarray(obsT[c])
        m["sdt"] = np.ascontiguousarray(SDt[c])
        m["dwt"] = np.ascontiguousarray(DWt[c])
        in_maps.append(m)
    return in_maps


LAST_EXEC_NS = None
TRACE = False


def _run(inputs, cfg):
    global LAST_EXEC_NS
    nc = build_nc(cfg)
    in_maps = preprocess(inputs, cfg)
    res = run_bass_kernel_spmd(
        nc, in_maps, core_ids=list(range(NCORES)), trace=TRACE
    )
    LAST_EXEC_NS = res.exec_time_ns
    bg = cfg["bg"]
    outs = [np.asarray(res.results[c]["out"], np.float32).reshape(bg, N) for c in range(NCORES)]
    full = np.concatenate(outs, 0)  # [B, 9]
    return np.ascontiguousarray(full[:, :8])


def kernel(**inputs) -> np.ndarray:
    return _run(inputs, CFG_FULL)


# revision 20
# speedup vs baseline: 1.2760x; 1.0119x over previous
"""Trainium2 Bass kernel for nn_MessagePassingGNN (B=8192 graphs, N=9 nodes,
16 edges + 9 self-loops per graph, 4 message-passing steps + GRU, decoder).

Strategy (v2, fp8 edge path):
  - Data-parallel over batch: each of 8 cores gets 1024 graphs.
  - Graphs packed into blocks of 14 (126 nodes, 350 edges incl self-loops);
    gather/scatter are matmuls against host-precomputed one-hot incidence
    matrices stored in fp8.
  - Edge-space heavy matmuls run in fp8e4m3 with DoubleRow perf mode
    (2 k-tiles per instruction, 0.5 cyc/row):
      * gather: the dst-P/src-P halves are the two k-tiles -> one matmul
      * m2: K=256 as 2x128 k-tiles
      * w3: K=256 as 2x128 k-tiles
      * scatter: edge chunks (0:128, 128:256) as k-tiles + single for rest
  - P (x@W1), GRU, encoder, decoder stay bf16 for accuracy (fp8 end-to-end
    numpy sim: edge-fp8 rel err ~0.014 vs harness gate 2e-2).
  - PSUM->SBUF fp8 converts (P, m3) run on Vector (GpSimd cannot touch
    PSUM); activations write fp8 directly; the SBUF-only GRU elementwise
    chain runs on GpSimd as plain tensor-tensor ops.
  - GRU gates use true Sigmoid (same activation table set as Tanh, so no
    table reloads); msg_b3 folded into the GRU input bias host-side.
"""

import numpy as np

try:
    import concourse.bass as bass  # noqa: F401
except Exception:  # pragma: no cover
    import sys

    sys.path.insert(0, "/opt/trn_rl_repo")

import ml_dtypes
import concourse.bass as bass
import concourse.bacc as bacc
import concourse.mybir as mybir
from concourse.bass import MemorySpace
from concourse.bass_utils import run_bass_kernel_spmd
from concourse.tile import TileContext


BF16 = mybir.dt.bfloat16
FP8 = mybir.dt.float8e4
F32 = mybir.dt.float32
NPBF16 = ml_dtypes.bfloat16
NPFP8 = ml_dtypes.float8_e4m3
AF = mybir.ActivationFunctionType
ALU = mybir.AluOpType
DR = mybir.MatmulPerfMode.DoubleRow

N, F_IN, H, MH, STEPS = 9, 15, 128, 256, 4
E_PER = 16
EPG = E_PER + N  # 25 edges per graph incl self-loops
NCORES = 8
GPB = 14  # graphs per full block
NN = GPB * N  # 126 nodes per full block
NE = GPB * EPG  # 350 edges per full block
NEP = NE + 2  # fp8 tiles pad the edge dim to a 4-byte-aligned stride

# bias-pack column map
COL_ENC = 0
COL_B1 = lambda s, c: 1 + 2 * s + c
COL_B2 = lambda s, c: 9 + 2 * s + c
COL_BRZ = lambda s, g: 17 + 2 * s + g  # 0.5*(bi'+bh)[g*128:+128]
COL_BHN = lambda s: 25 + s  # bh[256:384]
COL_BIN = lambda s: 29 + s  # bi'[256:384]
COL_DB1 = lambda c: 33 + c
COL_DB2 = lambda c: 35 + c
COL_DB3 = 37
NBIAS = 38


def _derive(bg):
    nblk = bg // GPB
    tailg = bg - nblk * GPB
    totblk = nblk + (1 if tailg else 0)
    nnode = bg * N
    return dict(bg=bg, nblk=nblk, tailg=tailg, totblk=totblk, nnode=nnode)


CFG_FULL = _derive(1024)

_NC_CACHE = {}


WAVE = 12      # pairs interleaved per wave
SD_BUFS = 24
XP_BUFS = 48
ACT_BUFS = 12
PB_BUFS = 4
PB2_BUFS = 2


def build_nc(cfg, repeat=1):
    key = (cfg["bg"], repeat, WAVE, SD_BUFS, XP_BUFS, ACT_BUFS, PB_BUFS, PB2_BUFS)
    if key in _NC_CACHE:
        return _NC_CACHE[key]
    nblk, tailg, totblk, nnode = (
        cfg["nblk"],
        cfg["tailg"],
        cfg["totblk"],
        cfg["nnode"],
    )
    tnn, tne = tailg * N, tailg * EPG

    nc = bacc.Bacc("TRN2", target_bir_lowering=False, debug=False, num_devices=NCORES)

    obsT_d = nc.dram_tensor("obsT", [F_IN, nnode], BF16, kind="ExternalInput")
    sdt_d = nc.dram_tensor("sdt", [totblk, 128, 2, NEP], FP8, kind="ExternalInput")
    dwt_d = nc.dram_tensor("dwt", [totblk, 3, 128, 128], FP8, kind="ExternalInput")
    encw_d = nc.dram_tensor("encw", [F_IN, H], BF16, kind="ExternalInput")
    w1_d = nc.dram_tensor("w1", [STEPS, 2 * H, MH], BF16, kind="ExternalInput")
    w2_d = nc.dram_tensor("w2", [STEPS, MH, MH], FP8, kind="ExternalInput")
    w3_d = nc.dram_tensor("w3", [STEPS, MH, H], FP8, kind="ExternalInput")
    wi_d = nc.dram_tensor("wi", [STEPS, H, 3 * H], BF16, kind="ExternalInput")
    wh_d = nc.dram_tensor("wh", [STEPS, H, 3 * H], BF16, kind="ExternalInput")
    dw1_d = nc.dram_tensor("dw1", [H, MH], BF16, kind="ExternalInput")
    dw2_d = nc.dram_tensor("dw2", [MH, MH], BF16, kind="ExternalInput")
    dw3_d = nc.dram_tensor("dw3", [MH, 1], BF16, kind="ExternalInput")
    bias_d = nc.dram_tensor("biases", [128, NBIAS], F32, kind="ExternalInput")
    out_d = nc.dram_tensor("out", [1, nnode], F32, kind="ExternalOutput")

    NN2 = 2 * NN

    with TileContext(nc) as tc:
        with (
            tc.tile_pool(name="const", bufs=1) as constp,
            tc.tile_pool(name="sd", bufs=SD_BUFS) as sdp,
            tc.tile_pool(name="dw", bufs=SD_BUFS) as dwp,
            tc.tile_pool(name="xp", bufs=XP_BUFS) as xpp,
            tc.tile_pool(name="eact", bufs=ACT_BUFS) as eactp,
            tc.tile_pool(name="gact", bufs=ACT_BUFS) as gactp,
            tc.tile_pool(name="pb", bufs=PB_BUFS, space=MemorySpace.PSUM) as ppb,
            tc.tile_pool(name="pb2", bufs=PB2_BUFS, space=MemorySpace.PSUM) as ppb2,
        ):
            obs_t = constp.tile([F_IN, nnode], BF16, tag="obs")
            nc.sync.dma_start(obs_t[:], obsT_d[:])
            encw_t = constp.tile([F_IN, H], BF16, tag="encw")
            nc.sync.dma_start(encw_t[:], encw_d[:])
            w1_t = constp.tile([128, STEPS, 2, MH], BF16, tag="w1")
            nc.sync.dma_start(
                w1_t[:], w1_d.rearrange("s (kc p) m -> p s kc m", p=128)
            )
            w2_t = constp.tile([128, STEPS, 2, MH], FP8, tag="w2")
            nc.sync.dma_start(
                w2_t[:], w2_d.rearrange("s (kc p) m -> p s kc m", p=128)
            )
            w3_t = constp.tile([128, STEPS, 2, H], FP8, tag="w3")
            nc.sync.dma_start(
                w3_t[:], w3_d.rearrange("s (kc p) m -> p s kc m", p=128)
            )
            wi_t = constp.tile([128, STEPS, 3 * H], BF16, tag="wi")
            nc.sync.dma_start(wi_t[:], wi_d.rearrange("s p m -> p s m"))
            wh_t = constp.tile([128, STEPS, 3 * H], BF16, tag="wh")
            nc.sync.dma_start(wh_t[:], wh_d.rearrange("s p m -> p s m"))
            dw1_t = constp.tile([128, MH], BF16, tag="dw1")
            nc.sync.dma_start(dw1_t[:], dw1_d[:])
            dw2_t = constp.tile([128, 2, MH], BF16, tag="dw2")
            nc.sync.dma_start(dw2_t[:], dw2_d.rearrange("(kc p) m -> p kc m", p=128))
            dw3_t = constp.tile([128, 2, 1], BF16, tag="dw3")
            nc.sync.dma_start(dw3_t[:], dw3_d.rearrange("(kc p) m -> p kc m", p=128))
            bias_t = constp.tile([128, NBIAS], F32, tag="bias")
            nc.sync.dma_start(bias_t[:], bias_d[:])

            def bcol(c):
                return bias_t[:, c : c + 1]

            tot = cfg["totblk"]
            pairs = [tuple(range(kk, min(kk + 2, tot))) for kk in range(0, tot, 2)]

            def geom(k):
                full = k < nblk
                nn = NN if full else tnn
                ne = NE if full else tne
                ecs = [(0, 128), (128, 128), (256, 94)] if full else [(0, tne)]
                return nn, ne, ecs

            class Ctx:
                pass

            def ph_load(cx):
                cx.sds, cx.dws = [], []
                for bi, k in enumerate(cx.pr):
                    nn, ne, ecs = cx.geos[bi]
                    sd = sdp.tile([128, 2, NEP], FP8, tag="sd", name="sd")
                    if k < nblk:
                        nc.sync.dma_start(sd[:, :, :], sdt_d[k])
                    else:
                        nc.sync.dma_start(
                            sd[:nn, :, :ne], sdt_d[k, :nn, :, :ne]
                        )
                    dwti = dwp.tile([128, 3, 128], FP8, tag="dw", name="dw")
                    nch = len(ecs)
                    nc.sync.dma_start(
                        dwti[:, :nch, :nn],
                        dwt_d[k, :nch, :, :nn].rearrange("c p f -> p c f"),
                    )
                    cx.sds.append(sd)
                    cx.dws.append(dwti)

            def ph_enc(cx):
                penc = ppb.tile([128, 512], F32, tag="pb", name="penc")
                nc.tensor.matmul(
                    penc[:, : cx.npair], encw_t[:, :], obs_t[:, cx.pcols],
                    start=True, stop=True,
                )
                cx.xcur = xpp.tile([128, NN2], BF16, tag="xp", name="x0")
                nc.scalar.activation(
                    cx.xcur[:, : cx.npair], penc[:, : cx.npair], AF.Tanh,
                    bias=bcol(COL_ENC),
                )

            def ph_P(cx, s):
                # P = x @ W1-halves in node space (row layout, bf16), then
                # converted fp32->fp8 into SBUF on gpsimd for the gather.
                cx.psb = eactp.tile(
                    [128, 2, 2, 128], FP8, tag="psb", name="psb"
                )  # [part(node), h, mc, feat]
                for bi in range(len(cx.pr)):
                    nn = cx.geos[bi][0]
                    c0 = NN * bi
                    pq = ppb.tile([128, 512], F32, tag="pb", name="pq")
                    nc.tensor.matmul(
                        pq[:nn, :],
                        cx.xcur[:, c0 : c0 + nn],
                        w1_t[:, s, :, :],
                        start=True, stop=True,
                    )
                    if bi == 0:
                        nc.vector.tensor_copy(
                            cx.psb[:126].rearrange("p h m f -> p (h m f)"),
                            pq[:126, :],
                        )
                    else:
                        cx.psb2 = eactp.tile(
                            [128, 2, 2, 128], FP8, tag="psb2", name="psb2"
                        )
                        nn1 = cx.geos[1][0]
                        nc.vector.tensor_copy(
                            cx.psb2[:nn1].rearrange("p h m f -> p (h m f)"),
                            pq[:nn1, :],
                        )

            def ph_m1(cx, s):
                # gather: one DoubleRow matmul per (mc, blk): k-tiles are the
                # dst-P/src-P halves vs the dst/src one-hots.
                cx.m1sb = eactp.tile(
                    [128, 2, 2, NEP], FP8, tag="m1", name="m1sb"
                )  # [part(feat), mc, blk, edge]
                for mc in range(2):
                    pm = ppb2.tile([128, 1024], F32, tag="pb2", name="pm")
                    for bi in range(len(cx.pr)):
                        nn, ne, _ = cx.geos[bi]
                        kk = nn
                        o = 512 * bi
                        psb = cx.psb if bi == 0 else cx.psb2
                        nc.tensor.matmul(
                            pm[:, o : o + ne],
                            psb[:kk, :, mc, :],
                            cx.sds[bi][:kk, :, :ne],
                            start=True, stop=True,
                            perf_mode=DR,
                        )
                    if cx.uni:
                        ne = cx.geos[0][1]
                        nc.scalar.activation(
                            cx.m1sb[:, mc, :, :ne],
                            pm.rearrange("p (b f) -> p b f", b=2)[:, :, :ne],
                            AF.Tanh, bias=bcol(COL_B1(s, mc)),
                        )
                    else:
                        for bi in range(len(cx.pr)):
                            ne = cx.geos[bi][1]
                            nc.scalar.activation(
                                cx.m1sb[:, mc, bi, :ne],
                                pm[:, 512 * bi : 512 * bi + ne],
                                AF.Tanh, bias=bcol(COL_B1(s, mc)),
                            )

            def ph_m2(cx, s):
                cx.m2sb = eactp.tile([128, 2, 2, NEP], FP8, tag="m2", name="m2sb")
                for mc in range(2):
                    pm = ppb2.tile([128, 1024], F32, tag="pb2", name="pm2")
                    for bi in range(len(cx.pr)):
                        nn, ne, _ = cx.geos[bi]
                        o = 512 * bi
                        nc.tensor.matmul(
                            pm[:, o : o + ne],
                            w2_t[:, s, :, mc * 128 : mc * 128 + 128],
                            cx.m1sb[:, :, bi, :ne],
                            start=True, stop=True,
                            perf_mode=DR,
                        )
                    if cx.uni:
                        ne = cx.geos[0][1]
                        nc.scalar.activation(
                            cx.m2sb[:, mc, :, :ne],
                            pm.rearrange("p (b f) -> p b f", b=2)[:, :, :ne],
                            AF.Tanh, bias=bcol(COL_B2(s, mc)),
                        )
                    else:
                        for bi in range(len(cx.pr)):
                            ne = cx.geos[bi][1]
                            nc.scalar.activation(
                                cx.m2sb[:, mc, bi, :ne],
                                pm[:, 512 * bi : 512 * bi + ne],
                                AF.Tanh, bias=bcol(COL_B2(s, mc)),
                            )

            def ph_w3(cx, s):
                # W3 row-form + scatter into one pair tile:
                # per block bi at 512*bi: m3 chunks [0:384], aggr [384:384+nn]
                cx.m3sb = eactp.tile([128, 2, 3, 128], FP8, tag="m3r", name="m3sb")
                cx.aggp = gactp.tile([128, NN2], BF16, tag="aggr", name="aggp")
                aoff = 0
                for bi in range(len(cx.pr)):
                    nn, _, ecs = cx.geos[bi]
                    pg3 = ppb.tile([128, 512], F32, tag="pb", name="pg3")
                    for ci, (e0, el) in enumerate(ecs):
                        nc.tensor.matmul(
                            pg3[:el, 128 * ci : 128 * ci + 128],
                            cx.m2sb[:, :, bi, e0 : e0 + el],
                            w3_t[:, s, :, :],
                            start=True, stop=True,
                            perf_mode=DR,
                        )
                    nch = len(ecs)
                    nc.vector.tensor_copy(
                        cx.m3sb[:, bi, :nch, :].rearrange("p c f -> p (c f)"),
                        pg3[:, : 128 * nch],
                    )
                    if len(ecs) == 3:
                        # edges 0:256 as two DoubleRow k-tiles, 256:350 single
                        nc.tensor.matmul(
                            pg3[:, 384 : 384 + nn],
                            cx.m3sb[:, bi, 0:2, :],
                            cx.dws[bi][:, 0:2, :nn],
                            start=True, stop=False,
                            perf_mode=DR,
                        )
                        nc.tensor.matmul(
                            pg3[:, 384 : 384 + nn],
                            cx.m3sb[:94, bi, 2, :],
                            cx.dws[bi][:94, 2, :nn],
                            start=False, stop=True,
                        )
                    else:
                        (e0, el) = ecs[0]
                        nc.tensor.matmul(
                            pg3[:, 384 : 384 + nn],
                            cx.m3sb[:el, bi, 0, :],
                            cx.dws[bi][:el, 0, :nn],
                            start=True, stop=True,
                        )
                    nc.vector.tensor_copy(
                        cx.aggp[:, aoff : aoff + nn], pg3[:, 384 : 384 + nn]
                    )
                    aoff += nn

            def ph_gru_mm(grp, s):
                # kind-major emission across the group: consecutive matmuls
                # share their stationary operand so walrus ldw-opt drops the
                # redundant LDWEIGHTS.
                for cx in grp:
                    cx.pgr = ppb.tile([128, 512], F32, tag="pb", name="pgr")
                    cx.pgn = ppb.tile([128, 512], F32, tag="pb", name="pgn")
                # pgr: rz0@0, rz1@npair; pgn: gin@0, ghn@npair
                for g, wt in ((0, wi_t), (0, wh_t), (1, wi_t), (1, wh_t)):
                    for cx in grp:
                        mov = cx.aggp if wt is wi_t else cx.xcur
                        nc.tensor.matmul(
                            cx.pgr[:, g * cx.npair : (g + 1) * cx.npair],
                            wt[:, s, g * 128 : g * 128 + 128],
                            mov[:, : cx.npair],
                            start=(wt is wi_t), stop=(wt is wh_t),
                        )
                for cx in grp:
                    nc.tensor.matmul(
                        cx.pgn[:, : cx.npair],
                        wi_t[:, s, 256:384],
                        cx.aggp[:, : cx.npair],
                        start=True, stop=True,
                    )
                for cx in grp:
                    nc.tensor.matmul(
                        cx.pgn[:, cx.npair : 2 * cx.npair],
                        wh_t[:, s, 256:384],
                        cx.xcur[:, : cx.npair],
                        start=True, stop=True,
                    )

            def ph_gru(cx, s):
                npair = cx.npair
                pgr = cx.pgr
                pgn = cx.pgn
                # r/z via true sigmoid: same act table set as tanh
                # (sigmoid_and_others), so no table reload and the gate
                # algebra becomes pure tensor-tensor ops for GpSimd.
                rg = gactp.tile([128, NN2], BF16, tag="thr", name="rg")
                cx.zg = gactp.tile([128, NN2], BF16, tag="thz", name="zg")
                nc.scalar.activation(
                    rg[:, :npair], pgr[:, :npair], AF.Sigmoid,
                    bias=bcol(COL_BRZ(s, 0)),
                )
                nc.scalar.activation(
                    cx.zg[:, :npair], pgr[:, npair : 2 * npair], AF.Sigmoid,
                    bias=bcol(COL_BRZ(s, 1)),
                )
                hnp = gactp.tile([128, NN2], BF16, tag="hnp", name="hnp", bufs=6)
                nc.vector.tensor_scalar_add(
                    hnp[:, :npair],
                    pgn[:, npair : 2 * npair],
                    bcol(COL_BHN(s)),
                )
                rhn = gactp.tile([128, NN2], BF16, tag="rhn", name="rhn", bufs=6)
                nc.gpsimd.tensor_mul(
                    rhn[:, :npair], rg[:, :npair], hnp[:, :npair]
                )
                cx.tn = gactp.tile([128, NN2], BF16, tag="tn", name="tn", bufs=6)
                nc.vector.scalar_tensor_tensor(
                    cx.tn[:, :npair], pgn[:, :npair],
                    bcol(COL_BIN(s)), rhn[:, :npair],
                    op0=ALU.add, op1=ALU.add,
                )

            def ph_xupd(cx, s):
                npair = cx.npair
                ngate = gactp.tile([128, NN2], BF16, tag="ng", name="ng")
                nc.scalar.activation(ngate[:, :npair], cx.tn[:, :npair], AF.Tanh)
                # x' = n + z*(x-n)
                d_ = gactp.tile([128, NN2], BF16, tag="d", name="d_", bufs=6)
                nc.gpsimd.tensor_sub(
                    d_[:, :npair], cx.xcur[:, :npair], ngate[:, :npair]
                )
                w_ = gactp.tile([128, NN2], BF16, tag="w", name="w_", bufs=6)
                nc.gpsimd.tensor_mul(
                    w_[:, :npair], cx.zg[:, :npair], d_[:, :npair]
                )
                xnxt = xpp.tile([128, NN2], BF16, tag="xp", name="xn")
                nc.gpsimd.tensor_add(
                    xnxt[:, :npair], w_[:, :npair], ngate[:, :npair]
                )
                cx.xcur = xnxt

            def ph_dec1(grp):
                for cx in grp:
                    cx.pd1 = ppb.tile([128, 512], F32, tag="pb", name="pd1")
                    cx.d1sb = gactp.tile([128, 2, NN2], BF16, tag="d1", name="d1sb")
                for mc in range(2):
                    for cx in grp:
                        nc.tensor.matmul(
                            cx.pd1[:, 252 * mc : 252 * mc + cx.npair],
                            dw1_t[:, mc * 128 : mc * 128 + 128],
                            cx.xcur[:, : cx.npair],
                            start=True, stop=True,
                        )
                for cx in grp:
                    for mc in range(2):
                        nc.scalar.activation(
                            cx.d1sb[:, mc, : cx.npair],
                            cx.pd1[:, 252 * mc : 252 * mc + cx.npair],
                            AF.Tanh, bias=bcol(COL_DB1(mc)),
                        )

            def ph_dec2(grp):
                for cx in grp:
                    cx.pd2 = ppb.tile([128, 512], F32, tag="pb", name="pd2")
                    cx.d2sb = gactp.tile([128, 2, NN2], BF16, tag="d2", name="d2sb")
                for mc in range(2):
                    for kc in range(2):
                        for cx in grp:
                            nc.tensor.matmul(
                                cx.pd2[:, 252 * mc : 252 * mc + cx.npair],
                                dw2_t[:, kc, mc * 128 : mc * 128 + 128],
                                cx.d1sb[:, kc, : cx.npair],
                                start=(kc == 0), stop=(kc == 1),
                            )
                for cx in grp:
                    for mc in range(2):
                        nc.scalar.activation(
                            cx.d2sb[:, mc, : cx.npair],
                            cx.pd2[:, 252 * mc : 252 * mc + cx.npair],
                            AF.Tanh, bias=bcol(COL_DB2(mc)),
                        )
                for cx in grp:
                    cx.pd3 = ppb.tile([128, 512], F32, tag="pb", name="pd3")
                for kc in range(2):
                    for cx in grp:
                        nc.tensor.matmul(
                            cx.pd3[:1, : cx.npair], dw3_t[:, kc, :],
                            cx.d2sb[:, kc, : cx.npair],
                            start=(kc == 0), stop=(kc == 1),
                        )
                for cx in grp:
                    outp = gactp.tile([1, NN2], F32, tag="outp", name="outp", bufs=6)
                    nc.vector.tensor_scalar_add(
                        outp[:, : cx.npair], cx.pd3[:1, : cx.npair],
                        bias_t[0:1, COL_DB3 : COL_DB3 + 1],
                    )
                    nc.sync.dma_start(out_d[:, cx.pcols], outp[:1, : cx.npair])

            for _rep in range(repeat):
                allpairs = list(pairs)
                waves = [
                    allpairs[i : i + WAVE] for i in range(0, len(allpairs), WAVE)
                ]
                for wv in waves:
                    cxs = []
                    for pr in wv:
                        cx = Ctx()
                        cx.pr = pr
                        cx.geos = [geom(k) for k in pr]
                        cx.uni = len(pr) == 2 and cx.geos[0] == cx.geos[1]
                        cx.col0 = NN * pr[0]
                        cx.npair = sum(g[0] for g in cx.geos)
                        cx.pcols = slice(cx.col0, cx.col0 + cx.npair)
                        cxs.append(cx)
                    for cx in cxs:
                        ph_load(cx)
                    for cx in cxs:
                        ph_enc(cx)
                    for s in range(STEPS):
                        for ph in (ph_P, ph_m1, ph_m2, ph_w3):
                            for cx in cxs:
                                ph(cx, s)
                        for i in range(0, len(cxs), 2):
                            grp = cxs[i : i + 2]
                            ph_gru_mm(grp, s)
                            for cx in grp:
                                ph_gru(cx, s)
                        for cx in cxs:
                            ph_xupd(cx, s)
                    for i in range(0, len(cxs), 2):
                        ph_dec1(cxs[i : i + 2])
                    for i in range(0, len(cxs), 2):
                        ph_dec2(cxs[i : i + 2])

    nc.compile()
    _NC_CACHE[key] = nc
    return nc


def preprocess(inputs, cfg):
    bg, nblk, tailg, totblk, nnode = (
        cfg["bg"], cfg["nblk"], cfg["tailg"], cfg["totblk"], cfg["nnode"],
    )
    b = bg * NCORES
    obs = np.asarray(inputs["obs"], np.float32)
    edges = np.asarray(inputs["edges"], np.int64)

    # one-hot incidence per graph
    src = edges[:, 0, :]
    dst = edges[:, 1, :]
    loops = np.broadcast_to(np.arange(N, dtype=np.int64), (b, N))
    src_all = np.concatenate([src, loops], 1)  # [b, 25]
    dst_all = np.concatenate([dst, loops], 1)
    nod = np.arange(N, dtype=np.int64)
    Sg = (src_all[:, None, :] == nod[None, :, None]).astype(np.float32)  # [b,9,25]
    Dg = (dst_all[:, None, :] == nod[None, :, None]).astype(np.float32)  # [b,9,25]
    deg = Dg.sum(2)  # [b, 9] >= 1
    Dw = Dg.transpose(0, 2, 1) / deg[:, None, :]  # [b, 25, 9]

    SDt = np.zeros((NCORES, totblk, 128, 2, NEP), NPFP8)
    DWf = np.zeros((NCORES, totblk, 384, 128), np.float32)
    Sg_ = Sg.reshape(NCORES, bg, N, EPG)
    Dg_ = Dg.reshape(NCORES, bg, N, EPG)
    Dw_ = Dw.reshape(NCORES, bg, EPG, N)
    nmain = nblk * GPB
    Sm = Sg_[:, :nmain].reshape(NCORES, nblk, GPB, N, EPG)
    Dm = Dg_[:, :nmain].reshape(NCORES, nblk, GPB, N, EPG)
    Wm = Dw_[:, :nmain].reshape(NCORES, nblk, GPB, EPG, N)
    for i in range(GPB):
        r = slice(N * i, N * i + N)
        c = slice(EPG * i, EPG * i + EPG)
        SDt[:, :nblk, r, 0, c] = Dm[:, :, i]  # dst-gather one-hot
        SDt[:, :nblk, r, 1, c] = Sm[:, :, i]  # src-gather one-hot
        DWf[:, :nblk, c, r] = Wm[:, :, i]
    for i in range(tailg):
        g = nmain + i
        r = slice(N * i, N * i + N)
        c = slice(EPG * i, EPG * i + EPG)
        SDt[:, nblk, r, 0, c] = Dg_[:, g]
        SDt[:, nblk, r, 1, c] = Sg_[:, g]
        DWf[:, nblk, c, r] = Dw_[:, g]
    DWt = DWf.reshape(NCORES, totblk, 3, 128, 128).astype(NPFP8)

    obsT = (
        obs.reshape(b, N, F_IN)
        .reshape(NCORES, nnode, F_IN)
        .transpose(0, 2, 1)
        .astype(NPBF16)
    )  # [8, 15, nnode]

    f32 = lambda x: np.asarray(x, np.float32)
    bf = lambda x: np.ascontiguousarray(f32(x)).astype(NPBF16)
    f8 = lambda x: np.ascontiguousarray(f32(x)).astype(NPFP8)

    biases = np.zeros((128, NBIAS), np.float32)
    biases[:, COL_ENC] = f32(inputs["enc_b"])
    gru_bi = f32(inputs["gru_bi"])
    gru_bh = f32(inputs["gru_bh"])
    msg_b3 = f32(inputs["msg_b3"])
    gru_Wi = f32(inputs["gru_Wi"])
    for s in range(STEPS):
        b1 = f32(inputs["msg_b1"][s])
        b2 = f32(inputs["msg_b2"][s])
        for c in range(2):
            biases[:, COL_B1(s, c)] = b1[128 * c : 128 * (c + 1)]
            biases[:, COL_B2(s, c)] = b2[128 * c : 128 * (c + 1)]
        bip = gru_bi[s] + msg_b3[s] @ gru_Wi[s]  # fold msg_b3 into GRU input bias
        for g in range(2):
            biases[:, COL_BRZ(s, g)] = (
                bip[128 * g : 128 * (g + 1)] + gru_bh[s][128 * g : 128 * (g + 1)]
            )
        biases[:, COL_BHN(s)] = gru_bh[s][256:384]
        biases[:, COL_BIN(s)] = bip[256:384]
    db1 = f32(inputs["dec_b1"])
    db2 = f32(inputs["dec_b2"])
    for c in range(2):
        biases[:, COL_DB1(c)] = db1[128 * c : 128 * (c + 1)]
        biases[:, COL_DB2(c)] = db2[128 * c : 128 * (c + 1)]
    biases[0, COL_DB3] = float(f32(inputs["dec_b3"])[0])

    shared = dict(
        encw=bf(inputs["enc_W"]),
        w1=bf(inputs["msg_W1"]),
        w2=f8(inputs["msg_W2"]),
        w3=f8(inputs["msg_W3"]),
        wi=bf(inputs["gru_Wi"]),
        wh=bf(inputs["gru_Wh"]),
        dw1=bf(inputs["dec_W1"]),
        dw2=bf(inputs["dec_W2"]),
        dw3=bf(inputs["dec_W3"]),
        biases=biases,
    )
    in_maps = []
    for c in range(NCORES):
        m = dict(shared)
        m["obsT"] = np.ascontiguousarray(obsT[c])
        m["sdt"] = np.ascontiguousarray(SDt[c])
        m["dwt"] = np.ascontiguousarray(DWt[c])
        in_maps.append(m)
    return in_maps


LAST_EXEC_NS = None
TRACE = False


def _run(inputs, cfg):
    global LAST_EXEC_NS
    nc = build_nc(cfg)
    in_maps = preprocess(inputs, cfg)
    res = run_bass_kernel_spmd(
        nc, in_maps, core_ids=list(range(NCORES)), trace=TRACE
    )
    LAST_EXEC_NS = res.exec_time_ns
    bg = cfg["bg"]
    outs = [np.asarray(res.results[c]["out"], np.float32).reshape(bg, N) for c in range(NCORES)]
    full = np.concatenate(outs, 0)  # [B, 9]
    return np.ascontiguousarray(full[:, :8])


def kernel(**inputs) -> np.ndarray:
    return _run(inputs, CFG_FULL)
